# revision 16
# baseline (speedup 1.0000x reference)
"""GAT (3-layer, PyG-style) forward on 8 Trainium2 NeuronCores.

v3 strategy:
  - Node space padded to 8*PC nodes; core c owns nodes [c*PC, (c+1)*PC).
  - Per layer, a DRAM "table" holds one 256B row per node:
      [h 64 | AD 4 | E1 4 | E2 4 | pad] (bf16), E1=exp(as), E2=exp(0.2*as).
    Each core computes ONLY its own block (fused into the previous layer's
    edge loop); blocks are AllGathered (in two row-halves, overlapping the
    edge loop tail) into the full table.
  - exp(leaky_relu(as+ad)) == max(exp(as)exp(ad), exp(.2as)exp(.2ad)), so
    the edge phase needs no leaky/exp at all: gather rows, multiply the
    E12 columns by the dst's D12=(exp(ad),exp(.2ad)), take the max of the
    halves -> per-edge softmax numerators, one strided reduce per tile for
    the denominator, value mult (bf16) + one strided reduce per tile for
    the aggregation. Self contributions precomputed in bulk per layer.
  - Per-pair post-processing (softmax divide, bias, elu, head-mean,
    transpose + next-layer matmul) batches both tiles of a pair.
  - Final: per-tile pooling matmul into PSUM, AllReduce, MLP head.
"""

import sys

sys.path.insert(0, "/opt/trn_rl_repo")

import numpy as np
import ml_dtypes

BF16 = ml_dtypes.bfloat16

MAXJ_CALL = 28  # <=3584 idx per dma_gather (ring 4096 w/ 64KB scratch)
NQ = 4          # SWDGE queues


# ----------------------------------------------------------------- host prep

def _prep(x, edge_index, batch, n_graphs):
    """Graph preprocessing. Returns a dict of host arrays + structure."""
    N = x.shape[0]
    NC = 8
    PC = int(np.ceil(N / NC / 128)) * 128          # nodes per core (padded)
    NSTAR = NC * PC
    TILES = PC // 128
    BLKROWS = PC + 1                               # +1 dummy row per core blk
    WIN = 2 * BLKROWS                              # gather window (2 blocks)
    assert WIN <= 32767

    src = edge_index[0].astype(np.int64)
    dst = edge_index[1].astype(np.int64)

    core_of = np.arange(NSTAR) // PC               # orig id -> core
    win_of = (core_of // 2).astype(np.int64)       # orig id -> window

    # per-dst in-degree per window (real edges only; self-loops added densely)
    degw = np.zeros((NSTAR, 4), np.int64)
    np.add.at(degw, (dst, win_of[src]), 1)

    # --- cluster nodes into tiles by window-degree profile ---------------
    rank_of = np.empty(NSTAR, np.int64)
    tile_K = np.zeros((NC, TILES, 4), np.int64)
    for c in range(NC):
        ids = np.arange(c * PC, (c + 1) * PC)
        prof = degw[ids]
        order = np.lexsort((prof[:, 3], prof[:, 2], prof[:, 1], prof[:, 0],
                            prof.max(1), -prof.sum(1)))
        rank_of[ids[order]] = np.arange(PC)
        ps = prof[order]
        tile_K[c] = np.array([ps[t * 128:(t + 1) * 128].max(0)
                              for t in range(TILES)])

    # cross-core round matching: sort tiles by (max desc, total desc), then
    # "snake" positions so pair (2i,2i+1) = sorted ranks (i, TILES-1-i) —
    # balances per-pair gather sizes while keeping pair tiles adjacent.
    snake = np.empty(TILES, np.int64)
    for s in range(TILES):
        snake[s] = 2 * s if s < (TILES + 1) // 2 else 2 * (TILES - 1 - s) + 1
    for c in range(NC):
        t_order = np.lexsort((-tile_K[c].sum(1), -tile_K[c].max(1)))
        newpos = np.empty(TILES, np.int64)
        newpos[t_order] = snake
        ids = np.arange(c * PC, (c + 1) * PC)
        r = rank_of[ids]
        rank_of[ids] = newpos[r // 128] * 128 + (r % 128)
        inv_snake = np.empty(TILES, np.int64)
        inv_snake[newpos[t_order]] = t_order
        tile_K[c] = tile_K[c][inv_snake]

    K_round = tile_K.max(axis=0)                   # [TILES, 4]

    table_row = core_of * BLKROWS + rank_of

    # --- slot lists --------------------------------------------------------
    dcore = core_of[dst]
    drank = rank_of[dst]
    dwin = win_of[src]
    order = np.lexsort((dwin, drank, dcore))
    src_o, dst_o = src[order], dst[order]
    dcore_o, drank_o, dwin_o = dcore[order], drank[order], dwin[order]
    loc_o = table_row[src_o] - dwin_o * WIN        # window-local row idx
    assert loc_o.min() >= 0 and loc_o.max() < WIN

    DUMMY_LOC = PC                                  # same local idx all windows
    slots = []
    for c in range(NC):
        core_slots = []
        for t in range(TILES):
            wslots = []
            for s in range(4):
                K = int(K_round[t, s])
                arr = np.full((128, K), DUMMY_LOC, np.int16) if K else \
                    np.zeros((128, 0), np.int16)
                wslots.append(arr)
            core_slots.append(wslots)
        slots.append(core_slots)
    kfill = np.zeros((NSTAR, 4), np.int64)
    p_all = drank_o % 128
    t_all = drank_o // 128
    for i in range(len(src_o)):
        c = dcore_o[i]
        t = t_all[i]
        s = dwin_o[i]
        p = p_all[i]
        k = kfill[dst_o[i], s]
        slots[c][t][s][p, k] = loc_o[i]
        kfill[dst_o[i], s] = k + 1

    # --- pair-merged gather call structure (uniform across cores) --------
    NPAIR = (TILES + 1) // 2
    calls = []            # (pr, s, pair_joff, cj)
    pair_J = np.zeros(NPAIR, np.int64)
    tile_rng = [[None] * 4 for _ in range(TILES)]   # (pair_joff, K) per window
    for pr in range(NPAIR):
        rA, rB = 2 * pr, min(2 * pr + 1, TILES - 1)
        single = rB == rA
        joff = 0
        for s in range(4):
            KA = int(K_round[rA, s])
            KB = 0 if single else int(K_round[rB, s])
            tile_rng[rA][s] = (joff, KA)
            if not single:
                tile_rng[rB][s] = (joff + KA, KB)
            K = KA + KB
            o = 0
            while o < K:
                cj = min(MAXJ_CALL, K - o)
                calls.append((pr, s, joff + o, cj))
                o += cj
            joff += K
        pair_J[pr] = joff

    # --- int16 wrapped idx arrays per core --------------------------------
    def wrap16(ix):                                 # [n] -> [128, n//16]
        a = ix.reshape(-1, 16).T
        return np.tile(a, (8, 1))

    tile_cols = []
    off = 0
    for pr in range(NPAIR):
        ncols = int(128 * pair_J[pr]) // 16
        tile_cols.append((off, ncols))
        off += ncols

    def pair_blocks(c, pr, s):
        rA, rB = 2 * pr, min(2 * pr + 1, TILES - 1)
        bA = slots[c][rA][s]
        if rB == rA:
            return bA
        return np.concatenate([bA, slots[c][rB][s]], axis=1)

    idx_cores = []
    for c in range(NC):
        parts = []
        for (pr, s, joff, cj) in calls:
            base = tile_rng[2 * pr][s][0]
            js = joff - base
            blk = pair_blocks(c, pr, s)[:, js:js + cj]   # [128, cj]
            ix = blk.T.reshape(-1).astype(np.int16)
            parts.append(wrap16(ix).astype(np.int16))
        idx_cores.append(np.concatenate(parts, axis=1)
                         if parts else np.zeros((128, 0), np.int16))
    idx_all = np.stack(idx_cores)                    # [NC, 128, TOTC]

    pi_of = core_of * PC + rank_of
    inv_pi = np.empty(NSTAR, np.int64)
    inv_pi[pi_of] = np.arange(NSTAR)

    batch_full = np.full(NSTAR, -1, np.int64)
    batch_full[:N] = batch
    goh = np.zeros((NC, PC, n_graphs), np.float32)
    for c in range(NC):
        b = batch_full[inv_pi[c * PC:(c + 1) * PC]]
        valid = b >= 0
        goh[c, np.arange(PC)[valid], b[valid]] = 1.0
    counts = np.maximum(np.bincount(batch, minlength=n_graphs), 1.0)

    return dict(
        N=N, NC=NC, PC=PC, NSTAR=NSTAR, TILES=TILES, BLKROWS=BLKROWS,
        WIN=WIN, K_round=K_round, calls=calls,
        tile_cols=tile_cols, idx_all=idx_all, inv_pi=inv_pi,
        goh=goh, counts=counts, DUMMY_LOC=DUMMY_LOC,
        NPAIR=NPAIR, pair_J=pair_J, tile_rng=tile_rng,
    )


def _augment_w(W, a_s, a_d, heads=4, hid=16):
    """[F, H*C] weights -> [F, 72] augmented (bf16): [W | Wad | Was]."""
    F = W.shape[0]
    Wr = W.reshape(F, heads, hid)
    was = np.einsum("fhc,hc->fh", Wr, a_s)
    wad = np.einsum("fhc,hc->fh", Wr, a_d)
    out = np.concatenate([W, wad, was], axis=1).astype(np.float32)
    return out.astype(BF16)


# ------------------------------------------------------------- kernel build

def _build(meta, n_graphs, f_in, run_layers=3):
    import concourse.bass as bass
    import concourse.tile as tile
    from concourse import bacc, mybir
    from concourse.masks import make_identity

    NC, PC, TILES = meta["NC"], meta["PC"], meta["TILES"]
    BLKROWS, WIN = meta["BLKROWS"], meta["WIN"]
    TROWS = NC * BLKROWS
    calls, tile_cols = meta["calls"], meta["tile_cols"]
    NPAIR, pair_J, tile_rng = meta["NPAIR"], meta["pair_J"], meta["tile_rng"]
    TOTC = meta["idx_all"].shape[2]
    G = n_graphs
    f32, bf16, i16 = mybir.dt.float32, mybir.dt.bfloat16, mybir.dt.int16
    AX, ALU = mybir.AxisListType, mybir.AluOpType
    ACT = mybir.ActivationFunctionType

    nc = bacc.Bacc(None, target_bir_lowering=False, debug=False,
                   num_devices=NC, num_swdge_queues=NQ,
                   dynamic_dma_scratch_size=65536)

    # ---- I/O ----
    xT = nc.dram_tensor("xT", [f_in, PC], bf16, kind="ExternalInput")
    idx_in = nc.dram_tensor("idx", [128, TOTC], i16, kind="ExternalInput")
    goh_in = nc.dram_tensor("goh", [PC, G], bf16, kind="ExternalInput")
    w1 = nc.dram_tensor("w1", [f_in, 72], bf16, kind="ExternalInput")
    w2 = nc.dram_tensor("w2", [64, 72], bf16, kind="ExternalInput")
    w3 = nc.dram_tensor("w3", [16, 72], bf16, kind="ExternalInput")
    b1r = nc.dram_tensor("b1r", [128, 128], f32, kind="ExternalInput")
    b2r = nc.dram_tensor("b2r", [128, 32], f32, kind="ExternalInput")
    b3r = nc.dram_tensor("b3r", [128, 32], f32, kind="ExternalInput")
    cntr = nc.dram_tensor("cntr", [16, G], f32, kind="ExternalInput")
    statsT = nc.dram_tensor("statsT", [16, G], f32, kind="ExternalInput")
    fw1 = nc.dram_tensor("fw1", [32, 32], f32, kind="ExternalInput")
    fb1 = nc.dram_tensor("fb1", [32, 1], f32, kind="ExternalInput")
    fw2 = nc.dram_tensor("fw2", [32, 16], f32, kind="ExternalInput")
    fb2 = nc.dram_tensor("fb2", [16, 1], f32, kind="ExternalInput")
    fw3 = nc.dram_tensor("fw3", [16, 1], f32, kind="ExternalInput")
    fb3 = nc.dram_tensor("fb3", [1, 1], f32, kind="ExternalInput")
    dumr = nc.dram_tensor("dumr", [1, 128], bf16, kind="ExternalInput")
    out_t = nc.dram_tensor("out", [1, G], f32, kind="ExternalOutput")

    LIN = [f_in, 64, 16]          # node-phase input width per layer
    LOUT = [64, 16, 16]           # edge-phase output width per layer

    HROWS = (TILES // 2) * 128    # first-half rows for the split AllGather

    with tile.TileContext(nc, num_cores=NC) as tc:
        with (
            tc.tile_pool(name="dram", bufs=1, space="DRAM") as dpool,
            tc.tile_pool(name="consts", bufs=1) as cpool,
            tc.tile_pool(name="nodein", bufs=1) as npool,
            tc.tile_pool(name="psum", bufs=2, space="PSUM") as ppool,
            tc.tile_pool(name="pst", bufs=2, space="PSUM") as ptpool,
            tc.tile_pool(name="mlpp", bufs=1, space="PSUM") as mpool,
            tc.tile_pool(name="stall", bufs=1) as stpool,
            tc.tile_pool(name="selfp", bufs=1) as sepool,
            tc.tile_pool(name="gat", bufs=2) as gpool,
            tc.tile_pool(name="vt", bufs=2) as vpool,
            tc.tile_pool(name="idxp", bufs=2) as ipool,
            tc.tile_pool(name="edge", bufs=3) as epool,
            tc.tile_pool(name="escr", bufs=2) as e1pool,
            tc.tile_pool(name="poolacc", bufs=1, space="PSUM") as papool,
            tc.tile_pool(name="head", bufs=1) as hpool,
        ):
            tabmine = [dpool.tile([BLKROWS, 128], bf16, tag=f"tm{l}",
                                  name=f"tabmine{l}") for l in range(3)]
            tables = [dpool.tile([TROWS, 128], bf16, tag=f"tab{l}",
                                 name=f"table{l}", addr_space="Shared")
                      for l in range(3)]
            cc_in = dpool.tile([16, G], f32, tag="ccin")
            cc_out = dpool.tile([16, G], f32, tag="ccout",
                                addr_space="Shared")

            ident = cpool.tile([128, 128], bf16)
            make_identity(nc, ident[:])
            wsb = []
            for l, wt in enumerate((w1, w2, w3)):
                t = cpool.tile([LIN[l], 72], bf16, tag=f"w{l}", name=f"wsb{l}")
                nc.sync.dma_start(t[:], wt[:, :])
                wsb.append(t)
            brep = []
            for l, (bt, w2_) in enumerate(((b1r, 128), (b2r, 32), (b3r, 32))):
                t = cpool.tile([128, w2_], f32, tag=f"b{l}", name=f"bsb{l}")
                nc.sync.dma_start(t[:], bt[:, :])
                brep.append(t)
            dum_sb = cpool.tile([1, 128], bf16)
            nc.sync.dma_start(dum_sb[:], dumr[:, :])

            # own-block staged rows [h64|AD4|E1 4|E2 4] = 76 cols per tile
            SW = 76
            st_bufs = [stpool.tile([128, TILES * SW], bf16, tag=f"sta{l}",
                                   name=f"stall{l}") for l in range(2)]
            st_all = [st_bufs[0], st_bufs[1], st_bufs[0]]

            pool_ps = papool.tile([16, G], f32)

            qctr = [0]

            def gather_queue():
                q = qctr[0] % NQ
                qctr[0] += 1
                return q

            def elu_inplace(x_ap, w):
                t1 = e1pool.tile([128, w], f32, tag="el1")
                t2 = e1pool.tile([128, w], f32, tag="el2")
                nc.vector.tensor_scalar_min(t1[:], x_ap, 0.0)
                nc.scalar.activation(t1[:], t1[:], ACT.Exp)
                nc.vector.tensor_scalar(out=t1[:], in0=t1[:], scalar1=-1.0,
                                        scalar2=0.0, op0=ALU.add, op1=ALU.min)
                nc.vector.tensor_scalar_max(t2[:], x_ap, 0.0)
                nc.vector.tensor_tensor(out=x_ap, in0=t1[:], in1=t2[:],
                                        op=ALU.add)

            def node_tile(l, r, lhsT_ap):
                """row_l[tile r] = [h|AD|E1|E2] -> st_all[l] slice + tabmine."""
                ps = ppool.tile([128, 72], f32, tag="nps")
                nc.tensor.matmul(ps[:], lhsT_ap, wsb[l][:],
                                 start=True, stop=True)
                sl = st_all[l][:, r * SW:(r + 1) * SW]
                nc.scalar.copy(sl[:, 0:68], ps[:, 0:68])
                nc.scalar.activation(sl[:, 68:72], ps[:, 68:72], ACT.Exp)
                nc.scalar.activation(sl[:, 72:76], ps[:, 68:72], ACT.Exp,
                                     scale=0.2)
                nc.scalar.dma_start(
                    tabmine[l][r * 128:(r + 1) * 128, 0:SW], sl)

            def ag_table(l):
                nc.scalar.dma_start(tabmine[l][PC:PC + 1, :], dum_sb[:])
                nc.gpsimd.collective_compute(
                    "AllGather", mybir.AluOpType.bypass,
                    replica_groups=[list(range(NC))],
                    ins=[tabmine[l].opt()], outs=[tables[l].opt()])

            # ---------------- layer 0 node phase (own block only) --------
            half_t = (TILES + 1) // 2
            for h in range(2):
                r0, r1 = h * half_t, min((h + 1) * half_t, TILES)
                if r0 >= r1:
                    continue
                xin = npool.tile([f_in, half_t * 128], bf16, tag="xin")
                nc.sync.dma_start(xin[:, 0:(r1 - r0) * 128],
                                  xT[:, r0 * 128:r1 * 128])
                for r in range(r0, r1):
                    node_tile(0, r, xin[:, (r - r0) * 128:(r - r0 + 1) * 128])
            ag_table(0)

            for l in range(run_layers):
                table = tables[l]
                W = LOUT[l]
                sa = st_all[l]
                sa3 = sa[:].rearrange("p (r c) -> p r c", c=SW)

                # ---- bulk self/dst precompute (whole own block) ----
                # D12[r] = [exp(ad) x4 | exp(.2 ad) x4]
                d12 = sepool.tile([128, TILES * 8], bf16, tag="d12")
                d123 = d12[:].rearrange("p (r v) -> p r v", v=8)
                nc.scalar.activation(d123[:, :, 0:4], sa3[:, :, 64:68],
                                     ACT.Exp)
                nc.scalar.activation(d123[:, :, 4:8], sa3[:, :, 64:68],
                                     ACT.Exp, scale=0.2)
                # es[r] = max(E1*D1, E2*D2)  (self-loop numerator)
                est = sepool.tile([128, TILES * 8], bf16, tag="est")
                est3 = est[:].rearrange("p (r v) -> p r v", v=8)
                nc.vector.tensor_tensor(out=est3[:, :, :],
                                        in0=sa3[:, :, 68:76],
                                        in1=d123[:, :, :], op=ALU.mult)
                es_all = sepool.tile([128, TILES * 4], f32, tag="esal")
                es3 = es_all[:].rearrange("p (r q) -> p r q", q=4)
                nc.vector.tensor_tensor(out=es3[:, :, :],
                                        in0=est3[:, :, 0:4],
                                        in1=est3[:, :, 4:8], op=ALU.max)
                sv_all = sepool.tile([128, TILES * 64], bf16, tag="sval")
                sv4 = sv_all[:].rearrange("p (r q c) -> p r q c", q=4, c=16)
                nc.vector.tensor_tensor(
                    out=sv4[:, :, :, :],
                    in0=sa3[:, :, 0:64].rearrange("p r (q c) -> p r q c",
                                                  c=16),
                    in1=es3.unsqueeze(3).to_broadcast([128, TILES, 4, 16]),
                    op=ALU.mult)

                # ---------------- edge phase ----------------
                for pr in range(NPAIR):
                    rA = 2 * pr
                    rB = min(2 * pr + 1, TILES - 1)
                    single = rB == rA
                    tiles_here = [rA] if single else [rA, rB]
                    npr = len(tiles_here)
                    J = int(pair_J[pr])
                    coff, ncols = tile_cols[pr]

                    # per-tile ranges: (window_major_off, tile_major_off, k)
                    rngs_of = {}
                    JT = {}
                    for r in tiles_here:
                        o_t = 0
                        rr = []
                        for s in range(4):
                            o_w, k = tile_rng[r][s]
                            if k > 0:
                                rr.append((o_w, o_t, k))
                                o_t += k
                        rngs_of[r] = rr
                        JT[r] = o_t
                    base_t = {}
                    bt = 0
                    for r in tiles_here:
                        base_t[r] = bt
                        bt += JT[r]

                    if J > 0:
                        it = ipool.tile([128, max(ncols, 1)], i16, tag="idx")
                        nc.sync.dma_start(it[:, 0:ncols],
                                          idx_in[:, coff:coff + ncols])
                        gat = gpool.tile([128, J * 128], bf16, tag="gat")
                        g3 = gat[:].rearrange("p (j e) -> p j e", e=128)
                        ccol = 0
                        for (pr2, s_, joff, cj) in calls:
                            if pr2 != pr:
                                continue
                            n_i = 128 * cj
                            nc.gpsimd.dma_gather(
                                g3[:, joff:joff + cj, :],
                                table[s_ * WIN:(s_ + 1) * WIN, :],
                                it[:, ccol:ccol + n_i // 16],
                                n_i, n_i, 128,
                                queue_num=gather_queue(),
                                single_packet=False)
                            ccol += n_i // 16

                        gatj = gat[:].rearrange("p (j c) -> p j c", c=128)
                        g4 = gat[:].rearrange("p (j q c) -> p j q c",
                                              q=8, c=16)

                        # ---- t12 = E12_src * D12_dst (tile-major j) ----
                        t12 = epool.tile([128, J * 8], bf16, tag="t12")
                        t123 = t12[:].rearrange("p (j v) -> p j v", v=8)
                        for r in tiles_here:
                            d12r = d12[:, r * 8:(r + 1) * 8]
                            for (o_w, o_t, k) in rngs_of[r]:
                                to = base_t[r] + o_t
                                nc.vector.tensor_tensor(
                                    out=t123[:, to:to + k, :],
                                    in0=gatj[:, o_w:o_w + k, 68:76],
                                    in1=d12r.unsqueeze(1).to_broadcast(
                                        [128, k, 8]),
                                    op=ALU.mult)
                        # ---- e = max of halves (j-major [j][q]) ----
                        e_b = epool.tile([128, J * 4], bf16, tag="eb")
                        eb3 = e_b[:].rearrange("p (j q) -> p j q", q=4)
                        nc.vector.tensor_tensor(out=eb3[:, :, :],
                                                in0=t123[:, :, 0:4],
                                                in1=t123[:, :, 4:8],
                                                op=ALU.max)
                        # ---- denom per tile: strided reduce over j ----
                        ebqj = e_b[:].rearrange("p (j q) -> p q j", q=4)
                        den = epool.tile([128, 8], f32, tag="den")
                        for i, r in enumerate(tiles_here):
                            jt = JT[r]
                            if jt == 0:
                                continue
                            b0 = base_t[r]
                            nc.vector.tensor_reduce(
                                den[:, 4 * i:4 * i + 4],
                                ebqj[:, :, b0:b0 + jt], AX.X, ALU.add)
                        # ---- weighted values: channel-major layout so the
                        # U-reduce reads contiguous j ----
                        v_t = vpool.tile([128, J * 64], bf16, tag="vt")
                        v4 = v_t[:].rearrange("p (q c j) -> p j q c",
                                              q=4, c=16)
                        for r in tiles_here:
                            for (o_w, o_t, k) in rngs_of[r]:
                                to = base_t[r] + o_t
                                nc.vector.tensor_tensor(
                                    out=v4[:, to:to + k, :, :],
                                    in0=g4[:, o_w:o_w + k, 0:4, :],
                                    in1=eb3[:, to:to + k, :].unsqueeze(
                                        3).to_broadcast([128, k, 4, 16]),
                                    op=ALU.mult)
                        v3c = v_t[:].rearrange("p (c j) -> p c j", c=64)

                    # ---- pair-level aggregation + post ----
                    U = epool.tile([128, npr * 64], f32, tag="U")
                    dful = epool.tile([128, npr * 4], f32, tag="dful")
                    for i, r in enumerate(tiles_here):
                        jt = JT[r] if J > 0 else 0
                        if jt > 0:
                            b0 = base_t[r]
                            nc.vector.tensor_reduce(
                                U[:, 64 * i:64 * i + 64],
                                v3c[:, :, b0:b0 + jt], AX.X, ALU.add)
                    all_j = J > 0 and all(JT[r] > 0 for r in tiles_here)
                    if all_j:
                        # contiguous pair slices (rB == rA + 1)
                        nc.vector.tensor_tensor(
                            out=U[:], in0=U[:],
                            in1=sv_all[:, rA * 64:(rA + npr) * 64],
                            op=ALU.add)
                        nc.vector.tensor_tensor(
                            out=dful[:], in0=den[:, 0:npr * 4],
                            in1=es_all[:, rA * 4:(rA + npr) * 4],
                            op=ALU.add)
                    else:
                        for i, r in enumerate(tiles_here):
                            jt = JT[r] if J > 0 else 0
                            if jt > 0:
                                nc.vector.tensor_tensor(
                                    out=U[:, 64 * i:64 * i + 64],
                                    in0=U[:, 64 * i:64 * i + 64],
                                    in1=sv_all[:, r * 64:(r + 1) * 64],
                                    op=ALU.add)
                                nc.vector.tensor_tensor(
                                    out=dful[:, 4 * i:4 * i + 4],
                                    in0=den[:, 4 * i:4 * i + 4],
                                    in1=es_all[:, r * 4:(r + 1) * 4],
                                    op=ALU.add)
                            else:
                                nc.vector.tensor_copy(
                                    U[:, 64 * i:64 * i + 64],
                                    sv_all[:, r * 64:(r + 1) * 64])
                                nc.vector.tensor_copy(
                                    dful[:, 4 * i:4 * i + 4],
                                    es_all[:, r * 4:(r + 1) * 4])
                    recip = epool.tile([128, npr * 4], f32, tag="rec")
                    nc.vector.reciprocal(recip[:], dful[:])
                    if l > 0:
                        nc.vector.tensor_scalar_mul(recip[:], recip[:], 0.25)
                    o64 = epool.tile([128, npr * 64], f32, tag="o64")
                    nc.vector.tensor_tensor(
                        out=o64[:].rearrange("p (t q c) -> p t q c",
                                             q=4, c=16),
                        in0=U[:].rearrange("p (t q c) -> p t q c",
                                           q=4, c=16),
                        in1=recip[:].rearrange(
                            "p (t q) -> p t q", q=4).unsqueeze(
                            3).to_broadcast([128, npr, 4, 16]),
                        op=ALU.mult)
                    if l == 0:
                        nc.vector.tensor_tensor(
                            out=o64[:], in0=o64[:],
                            in1=brep[0][:, 0:npr * 64], op=ALU.add)
                        elu_inplace(o64[:], npr * 64)
                        xnext = epool.tile([128, npr * 64], bf16, tag="xn")
                        nc.vector.tensor_copy(xnext[:], o64[:])
                        WX = 64
                    else:
                        o16 = epool.tile([128, npr * 16], f32, tag="o16")
                        nc.vector.tensor_reduce(
                            o16[:],
                            o64[:].rearrange("p (t q c) -> p t c q",
                                             q=4, c=16),
                            AX.X, ALU.add)
                        nc.vector.tensor_tensor(
                            out=o16[:], in0=o16[:],
                            in1=brep[l][:, 0:npr * 16], op=ALU.add)
                        if l == 1:
                            elu_inplace(o16[:], npr * 16)
                        xnext = epool.tile([128, npr * 16], bf16, tag="xn16")
                        nc.vector.tensor_copy(xnext[:], o16[:])
                        WX = 16

                    if l < 2:
                        for i, r in enumerate(tiles_here):
                            pst = ptpool.tile([WX, 128], bf16, tag="pst")
                            nc.tensor.transpose(
                                out=pst[:], in_=xnext[:, WX * i:WX * (i + 1)],
                                identity=ident[:])
                            stt = epool.tile([WX, 128], bf16, tag="stt")
                            nc.scalar.copy(stt[:], pst[:])
                            if run_layers > l + 1:
                                node_tile(l + 1, r, stt[:])
                    else:
                        for i, r in enumerate(tiles_here):
                            gt = epool.tile([128, G], bf16, tag="goh")
                            nc.sync.dma_start(
                                gt[:], goh_in[r * 128:(r + 1) * 128, :])
                            nc.tensor.matmul(
                                pool_ps[:], xnext[:, 16 * i:16 * i + 16],
                                gt[:], start=(r == 0),
                                stop=(r == TILES - 1))

                if l < 2 and run_layers > l + 1:
                    ag_table(l + 1)

            # ---------------- pooling + MLP head ----------------
            if run_layers == 3:
                pooled = hpool.tile([16, G], f32, tag="pooled")
                nc.scalar.copy(pooled[:], pool_ps[:])
                nc.sync.dma_start(cc_in[:, :], pooled[:])
                nc.gpsimd.collective_compute(
                    "AllReduce", mybir.AluOpType.add,
                    replica_groups=[list(range(NC))],
                    ins=[cc_in.opt()], outs=[cc_out.opt()])
                zt = hpool.tile([32, G], f32, tag="zt")
                nc.sync.dma_start(zt[0:16, :], cc_out[:, :])
                cr = hpool.tile([16, G], f32, tag="cr")
                nc.sync.dma_start(cr[:], cntr[:, :])
                nc.vector.tensor_tensor(out=zt[0:16, :], in0=zt[0:16, :],
                                        in1=cr[:], op=ALU.mult)
                nc.sync.dma_start(zt[16:32, :], statsT[:, :])
                fw1s = hpool.tile([32, 32], f32, tag="fw1")
                nc.sync.dma_start(fw1s[:], fw1[:, :])
                fb1s = hpool.tile([32, 1], f32, tag="fb1")
                nc.sync.dma_start(fb1s[:], fb1[:, :])
                fw2s = hpool.tile([32, 16], f32, tag="fw2")
                nc.sync.dma_start(fw2s[:], fw2[:, :])
                fb2s = hpool.tile([16, 1], f32, tag="fb2")
                nc.sync.dma_start(fb2s[:], fb2[:, :])
                fw3s = hpool.tile([16, 1], f32, tag="fw3")
                nc.sync.dma_start(fw3s[:], fw3[:, :])
                fb3s = hpool.tile([1, 1], f32, tag="fb3")
                nc.sync.dma_start(fb3s[:], fb3[:, :])

                mp1 = mpool.tile([32, G], f32, tag="mp1")
                nc.tensor.matmul(mp1[:], fw1s[:], zt[:], start=True, stop=True)
                h1 = hpool.tile([32, G], f32, tag="h1")
                nc.scalar.activation(h1[:], mp1[:], ACT.Relu, bias=fb1s[:, 0:1])
                mp2 = mpool.tile([16, G], f32, tag="mp2")
                nc.tensor.matmul(mp2[:], fw2s[:], h1[:], start=True, stop=True)
                h2 = hpool.tile([16, G], f32, tag="h2")
                nc.scalar.activation(h2[:], mp2[:], ACT.Relu, bias=fb2s[:, 0:1])
                mp3 = mpool.tile([1, G], f32, tag="mp3")
                nc.tensor.matmul(mp3[:], fw3s[:], h2[:], start=True, stop=True)
                ot = hpool.tile([1, G], f32, tag="ot")
                nc.vector.tensor_tensor(
                    out=ot[:], in0=mp3[:],
                    in1=fb3s[:, 0:1].to_broadcast([1, G]), op=ALU.add)
                nc.sync.dma_start(out_t[:, :], ot[:])

    nc.finalize()
    return nc


# ------------------------------------------------------------------- driver

def run_gat(x, stats, W1, a1s, a1d, b1, W2, a2s, a2d, b2, W3, a3s, a3d, b3,
            fw1, fb1, fw2, fb2, fw3, fb3, edge_index, batch,
            trace=False, _cache={}):
    from concourse.bass_utils import run_bass_kernel_spmd

    x = np.asarray(x, np.float32)
    stats = np.asarray(stats, np.float32)
    n_graphs = stats.shape[0]
    f_in = x.shape[1]
    meta = _prep(x, np.asarray(edge_index), np.asarray(batch), n_graphs)
    NC, PC, NSTAR = meta["NC"], meta["PC"], meta["NSTAR"]

    nc = _build(meta, n_graphs, f_in)

    # host-side input prep
    inv_pi = meta["inv_pi"]
    xs = np.zeros((NSTAR, f_in), np.float32)
    xs[:x.shape[0]] = x
    xT_full = np.ascontiguousarray(xs[inv_pi].T).astype(BF16)  # [f_in, NSTAR]

    cntrep = np.tile((1.0 / meta["counts"]).astype(np.float32)[None, :],
                     (16, 1))
    in_common = dict(
        w1=_augment_w(np.asarray(W1, np.float32), np.asarray(a1s, np.float32),
                      np.asarray(a1d, np.float32)),
        w2=_augment_w(np.asarray(W2, np.float32), np.asarray(a2s, np.float32),
                      np.asarray(a2d, np.float32)),
        w3=_augment_w(np.asarray(W3, np.float32), np.asarray(a3s, np.float32),
                      np.asarray(a3d, np.float32)),
        b1r=np.tile(np.asarray(b1, np.float32)[None, :], (128, 2)),
        b2r=np.tile(np.asarray(b2, np.float32)[None, :], (128, 2)),
        b3r=np.tile(np.asarray(b3, np.float32)[None, :], (128, 2)),
        cntr=cntrep.astype(np.float32),
        statsT=np.ascontiguousarray(stats.T).astype(np.float32),
        fw1=np.asarray(fw1, np.float32),
        fb1=np.asarray(fb1, np.float32).reshape(32, 1),
        fw2=np.asarray(fw2, np.float32),
        fb2=np.asarray(fb2, np.float32).reshape(16, 1),
        fw3=np.asarray(fw3, np.float32),
        fb3=np.asarray(fb3, np.float32).reshape(1, 1),
        dumr=np.zeros((1, 128), np.float32).astype(BF16),
    )
    in_maps = []
    for c in range(NC):
        m = dict(in_common)
        m["xT"] = np.ascontiguousarray(xT_full[:, c * PC:(c + 1) * PC])
        m["idx"] = np.ascontiguousarray(meta["idx_all"][c])
        m["goh"] = meta["goh"][c].astype(BF16)
        in_maps.append(m)

    res = run_bass_kernel_spmd(nc, in_maps, list(range(NC)), trace=trace)
    out = res.results[0]["out"]                      # [1, G]
    return np.ascontiguousarray(out.T).astype(np.float32), res


def kernel(**inputs):
    out, _ = run_gat(**inputs)
    return out


# revision 19
# speedup vs baseline: 1.1544x; 1.1544x over previous
"""GAT (3-layer, PyG-style) forward on 8 Trainium2 NeuronCores.

v3 strategy:
  - Node space padded to 8*PC nodes; core c owns nodes [c*PC, (c+1)*PC).
  - Per layer, a DRAM "table" holds one 256B row per node:
      [h 64 | AD 4 | E1 4 | E2 4 | pad] (bf16), E1=exp(as), E2=exp(0.2*as).
    Each core computes ONLY its own block (fused into the previous layer's
    edge loop); blocks are AllGathered (in two row-halves, overlapping the
    edge loop tail) into the full table.
  - exp(leaky_relu(as+ad)) == max(exp(as)exp(ad), exp(.2as)exp(.2ad)), so
    the edge phase needs no leaky/exp at all: gather rows, multiply the
    E12 columns by the dst's D12=(exp(ad),exp(.2ad)), take the max of the
    halves -> per-edge softmax numerators, one strided reduce per tile for
    the denominator, value mult (bf16) + one strided reduce per tile for
    the aggregation. Self contributions precomputed in bulk per layer.
  - Per-pair post-processing (softmax divide, bias, elu, head-mean,
    transpose + next-layer matmul) batches both tiles of a pair.
  - Final: per-tile pooling matmul into PSUM, AllReduce, MLP head.
"""

import sys

sys.path.insert(0, "/opt/trn_rl_repo")

import numpy as np
import ml_dtypes

BF16 = ml_dtypes.bfloat16

MAXJ_CALL = 28  # <=3584 idx per dma_gather (ring 4096 w/ 64KB scratch)
NQ = 4          # SWDGE queues


# ----------------------------------------------------------------- host prep

def _prep(x, edge_index, batch, n_graphs):
    """Graph preprocessing. Returns a dict of host arrays + structure."""
    N = x.shape[0]
    NC = 8
    PC = int(np.ceil(N / NC / 128)) * 128          # nodes per core (padded)
    NSTAR = NC * PC
    TILES = PC // 128
    BLKROWS = PC + 1                               # +1 dummy row per core blk
    WIN = 2 * BLKROWS                              # gather window (2 blocks)
    assert WIN <= 32767

    src = edge_index[0].astype(np.int64)
    dst = edge_index[1].astype(np.int64)

    core_of = np.arange(NSTAR) // PC               # orig id -> core
    win_of = (core_of // 2).astype(np.int64)       # orig id -> window

    # per-dst in-degree per window (real edges only; self-loops added densely)
    degw = np.zeros((NSTAR, 4), np.int64)
    np.add.at(degw, (dst, win_of[src]), 1)

    # --- cluster nodes into tiles by window-degree profile ---------------
    rank_of = np.empty(NSTAR, np.int64)
    tile_K = np.zeros((NC, TILES, 4), np.int64)
    for c in range(NC):
        ids = np.arange(c * PC, (c + 1) * PC)
        prof = degw[ids]
        order = np.lexsort((prof[:, 3], prof[:, 2], prof[:, 1], prof[:, 0],
                            prof.max(1), -prof.sum(1)))
        rank_of[ids[order]] = np.arange(PC)
        ps = prof[order]
        tile_K[c] = np.array([ps[t * 128:(t + 1) * 128].max(0)
                              for t in range(TILES)])

    # cross-core round matching: sort tiles by (max desc, total desc), then
    # "snake" positions so pair (2i,2i+1) = sorted ranks (i, TILES-1-i) —
    # balances per-pair gather sizes while keeping pair tiles adjacent.
    snake = np.empty(TILES, np.int64)
    for s in range(TILES):
        snake[s] = 2 * s if s < (TILES + 1) // 2 else 2 * (TILES - 1 - s) + 1
    for c in range(NC):
        t_order = np.lexsort((-tile_K[c].sum(1), -tile_K[c].max(1)))
        newpos = np.empty(TILES, np.int64)
        newpos[t_order] = snake
        ids = np.arange(c * PC, (c + 1) * PC)
        r = rank_of[ids]
        rank_of[ids] = newpos[r // 128] * 128 + (r % 128)
        inv_snake = np.empty(TILES, np.int64)
        inv_snake[newpos[t_order]] = t_order
        tile_K[c] = tile_K[c][inv_snake]

    K_round = tile_K.max(axis=0)                   # [TILES, 4]

    table_row = core_of * BLKROWS + rank_of

    # --- slot lists --------------------------------------------------------
    dcore = core_of[dst]
    drank = rank_of[dst]
    dwin = win_of[src]
    order = np.lexsort((dwin, drank, dcore))
    src_o, dst_o = src[order], dst[order]
    dcore_o, drank_o, dwin_o = dcore[order], drank[order], dwin[order]
    loc_o = table_row[src_o] - dwin_o * WIN        # window-local row idx
    assert loc_o.min() >= 0 and loc_o.max() < WIN

    DUMMY_LOC = PC                                  # same local idx all windows
    slots = []
    for c in range(NC):
        core_slots = []
        for t in range(TILES):
            wslots = []
            for s in range(4):
                K = int(K_round[t, s])
                arr = np.full((128, K), DUMMY_LOC, np.int16) if K else \
                    np.zeros((128, 0), np.int16)
                wslots.append(arr)
            core_slots.append(wslots)
        slots.append(core_slots)
    kfill = np.zeros((NSTAR, 4), np.int64)
    p_all = drank_o % 128
    t_all = drank_o // 128
    for i in range(len(src_o)):
        c = dcore_o[i]
        t = t_all[i]
        s = dwin_o[i]
        p = p_all[i]
        k = kfill[dst_o[i], s]
        slots[c][t][s][p, k] = loc_o[i]
        kfill[dst_o[i], s] = k + 1

    # --- pair-merged gather call structure (uniform across cores) --------
    NPAIR = (TILES + 1) // 2
    calls = []            # (pr, s, pair_joff, cj)
    pair_J = np.zeros(NPAIR, np.int64)
    tile_rng = [[None] * 4 for _ in range(TILES)]   # (pair_joff, K) per window
    for pr in range(NPAIR):
        rA, rB = 2 * pr, min(2 * pr + 1, TILES - 1)
        single = rB == rA
        joff = 0
        for s in range(4):
            KA = int(K_round[rA, s])
            KB = 0 if single else int(K_round[rB, s])
            tile_rng[rA][s] = (joff, KA)
            if not single:
                tile_rng[rB][s] = (joff + KA, KB)
            K = KA + KB
            o = 0
            while o < K:
                cj = min(MAXJ_CALL, K - o)
                calls.append((pr, s, joff + o, cj))
                o += cj
            joff += K
        pair_J[pr] = joff

    # --- int16 wrapped idx arrays per core --------------------------------
    def wrap16(ix):                                 # [n] -> [128, n//16]
        a = ix.reshape(-1, 16).T
        return np.tile(a, (8, 1))

    tile_cols = []
    off = 0
    for pr in range(NPAIR):
        ncols = int(128 * pair_J[pr]) // 16
        tile_cols.append((off, ncols))
        off += ncols

    def pair_blocks(c, pr, s):
        rA, rB = 2 * pr, min(2 * pr + 1, TILES - 1)
        bA = slots[c][rA][s]
        if rB == rA:
            return bA
        return np.concatenate([bA, slots[c][rB][s]], axis=1)

    idx_cores = []
    for c in range(NC):
        parts = []
        for (pr, s, joff, cj) in calls:
            base = tile_rng[2 * pr][s][0]
            js = joff - base
            blk = pair_blocks(c, pr, s)[:, js:js + cj]   # [128, cj]
            ix = blk.T.reshape(-1).astype(np.int16)
            parts.append(wrap16(ix).astype(np.int16))
        idx_cores.append(np.concatenate(parts, axis=1)
                         if parts else np.zeros((128, 0), np.int16))
    idx_all = np.stack(idx_cores)                    # [NC, 128, TOTC]

    pi_of = core_of * PC + rank_of
    inv_pi = np.empty(NSTAR, np.int64)
    inv_pi[pi_of] = np.arange(NSTAR)

    batch_full = np.full(NSTAR, -1, np.int64)
    batch_full[:N] = batch
    goh = np.zeros((NC, PC, n_graphs), np.float32)
    for c in range(NC):
        b = batch_full[inv_pi[c * PC:(c + 1) * PC]]
        valid = b >= 0
        goh[c, np.arange(PC)[valid], b[valid]] = 1.0
    counts = np.maximum(np.bincount(batch, minlength=n_graphs), 1.0)

    return dict(
        N=N, NC=NC, PC=PC, NSTAR=NSTAR, TILES=TILES, BLKROWS=BLKROWS,
        WIN=WIN, K_round=K_round, calls=calls,
        tile_cols=tile_cols, idx_all=idx_all, inv_pi=inv_pi,
        goh=goh, counts=counts, DUMMY_LOC=DUMMY_LOC,
        NPAIR=NPAIR, pair_J=pair_J, tile_rng=tile_rng,
    )


def _augment_w(W, a_s, a_d, heads=4, hid=16):
    """[F, H*C] weights -> [F, 72] augmented (bf16): [W | Wad | Was]."""
    F = W.shape[0]
    Wr = W.reshape(F, heads, hid)
    was = np.einsum("fhc,hc->fh", Wr, a_s)
    wad = np.einsum("fhc,hc->fh", Wr, a_d)
    out = np.concatenate([W, wad, was], axis=1).astype(np.float32)
    return out.astype(BF16)


# ------------------------------------------------------------- kernel build

def _build(meta, n_graphs, f_in, run_layers=3):
    import concourse.bass as bass
    import concourse.tile as tile
    from concourse import bacc, mybir
    from concourse.masks import make_identity

    NC, PC, TILES = meta["NC"], meta["PC"], meta["TILES"]
    BLKROWS, WIN = meta["BLKROWS"], meta["WIN"]
    TROWS = NC * BLKROWS
    calls, tile_cols = meta["calls"], meta["tile_cols"]
    NPAIR, pair_J, tile_rng = meta["NPAIR"], meta["pair_J"], meta["tile_rng"]
    TOTC = meta["idx_all"].shape[2]
    G = n_graphs
    f32, bf16, i16 = mybir.dt.float32, mybir.dt.bfloat16, mybir.dt.int16
    AX, ALU = mybir.AxisListType, mybir.AluOpType
    ACT = mybir.ActivationFunctionType

    nc = bacc.Bacc(None, target_bir_lowering=False, debug=False,
                   num_devices=NC, num_swdge_queues=NQ,
                   dynamic_dma_scratch_size=65536)

    # ---- I/O ----
    xT = nc.dram_tensor("xT", [f_in, PC], bf16, kind="ExternalInput")
    idx_in = nc.dram_tensor("idx", [128, TOTC], i16, kind="ExternalInput")
    goh_in = nc.dram_tensor("goh", [PC, G], bf16, kind="ExternalInput")
    w1 = nc.dram_tensor("w1", [f_in, 72], bf16, kind="ExternalInput")
    w2 = nc.dram_tensor("w2", [64, 72], bf16, kind="ExternalInput")
    w3 = nc.dram_tensor("w3", [16, 72], bf16, kind="ExternalInput")
    b1r = nc.dram_tensor("b1r", [128, 128], f32, kind="ExternalInput")
    b2r = nc.dram_tensor("b2r", [128, 32], f32, kind="ExternalInput")
    b3r = nc.dram_tensor("b3r", [128, 32], f32, kind="ExternalInput")
    cntr = nc.dram_tensor("cntr", [16, G], f32, kind="ExternalInput")
    statsT = nc.dram_tensor("statsT", [16, G], f32, kind="ExternalInput")
    fw1 = nc.dram_tensor("fw1", [32, 32], f32, kind="ExternalInput")
    fb1 = nc.dram_tensor("fb1", [32, 1], f32, kind="ExternalInput")
    fw2 = nc.dram_tensor("fw2", [32, 16], f32, kind="ExternalInput")
    fb2 = nc.dram_tensor("fb2", [16, 1], f32, kind="ExternalInput")
    fw3 = nc.dram_tensor("fw3", [16, 1], f32, kind="ExternalInput")
    fb3 = nc.dram_tensor("fb3", [1, 1], f32, kind="ExternalInput")
    dumr = nc.dram_tensor("dumr", [1, 128], bf16, kind="ExternalInput")
    out_t = nc.dram_tensor("out", [1, G], f32, kind="ExternalOutput")

    LIN = [f_in, 64, 16]          # node-phase input width per layer
    LOUT = [64, 16, 16]           # edge-phase output width per layer

    HROWS = (TILES // 2) * 128    # first-half rows for the split AllGather

    with tile.TileContext(nc, num_cores=NC) as tc:
        with (
            tc.tile_pool(name="dram", bufs=1, space="DRAM") as dpool,
            tc.tile_pool(name="consts", bufs=1) as cpool,
            tc.tile_pool(name="nodein", bufs=1) as npool,
            tc.tile_pool(name="psum", bufs=2, space="PSUM") as ppool,
            tc.tile_pool(name="pst", bufs=2, space="PSUM") as ptpool,
            tc.tile_pool(name="mlpp", bufs=1, space="PSUM") as mpool,
            tc.tile_pool(name="stall", bufs=1) as stpool,
            tc.tile_pool(name="selfp", bufs=1) as sepool,
            tc.tile_pool(name="gat", bufs=2) as gpool,
            tc.tile_pool(name="vt", bufs=2) as vpool,
            tc.tile_pool(name="idxp", bufs=2) as ipool,
            tc.tile_pool(name="edge", bufs=3) as epool,
            tc.tile_pool(name="escr", bufs=2) as e1pool,
            tc.tile_pool(name="poolacc", bufs=1, space="PSUM") as papool,
            tc.tile_pool(name="head", bufs=1) as hpool,
        ):
            tabmine = [dpool.tile([BLKROWS, 128], bf16, tag=f"tm{l}",
                                  name=f"tabmine{l}") for l in range(3)]
            tables = [dpool.tile([TROWS, 128], bf16, tag=f"tab{l}",
                                 name=f"table{l}", addr_space="Shared")
                      for l in range(3)]
            cc_in = dpool.tile([16, G], f32, tag="ccin")
            cc_out = dpool.tile([16, G], f32, tag="ccout",
                                addr_space="Shared")

            ident = cpool.tile([128, 128], bf16)
            make_identity(nc, ident[:])
            wsb = []
            for l, wt in enumerate((w1, w2, w3)):
                t = cpool.tile([LIN[l], 72], bf16, tag=f"w{l}", name=f"wsb{l}")
                nc.sync.dma_start(t[:], wt[:, :])
                wsb.append(t)
            brep = []
            for l, (bt, w2_) in enumerate(((b1r, 128), (b2r, 32), (b3r, 32))):
                t = cpool.tile([128, w2_], f32, tag=f"b{l}", name=f"bsb{l}")
                nc.sync.dma_start(t[:], bt[:, :])
                brep.append(t)
            dum_sb = cpool.tile([1, 128], bf16)
            nc.sync.dma_start(dum_sb[:], dumr[:, :])

            # own-block staged rows [h64|AD4|E1 4|E2 4] = 76 cols per tile
            SW = 76
            st_bufs = [stpool.tile([128, TILES * SW], bf16, tag=f"sta{l}",
                                   name=f"stall{l}") for l in range(2)]
            st_all = [st_bufs[0], st_bufs[1], st_bufs[0]]

            pool_ps = papool.tile([16, G], f32)

            qctr = [0]

            def gather_queue():
                q = qctr[0] % NQ
                qctr[0] += 1
                return q

            def elu_inplace(x_ap, w):
                t1 = e1pool.tile([128, w], f32, tag="el1")
                t2 = e1pool.tile([128, w], f32, tag="el2")
                nc.vector.tensor_scalar_min(t1[:], x_ap, 0.0)
                nc.scalar.activation(t1[:], t1[:], ACT.Exp)
                nc.vector.tensor_scalar(out=t1[:], in0=t1[:], scalar1=-1.0,
                                        scalar2=0.0, op0=ALU.add, op1=ALU.min)
                nc.vector.tensor_scalar_max(t2[:], x_ap, 0.0)
                nc.vector.tensor_tensor(out=x_ap, in0=t1[:], in1=t2[:],
                                        op=ALU.add)

            def node_tile(l, r, lhsT_ap):
                """row_l[tile r] = [h|AD|E1|E2] -> st_all[l] slice + tabmine."""
                ps = ppool.tile([128, 72], f32, tag="nps")
                nc.tensor.matmul(ps[:], lhsT_ap, wsb[l][:],
                                 start=True, stop=True)
                sl = st_all[l][:, r * SW:(r + 1) * SW]
                nc.scalar.copy(sl[:, 0:68], ps[:, 0:68])
                nc.scalar.activation(sl[:, 68:72], ps[:, 68:72], ACT.Exp)
                nc.scalar.activation(sl[:, 72:76], ps[:, 68:72], ACT.Exp,
                                     scale=0.2)
                nc.scalar.dma_start(
                    tabmine[l][r * 128:(r + 1) * 128, 0:SW], sl)

            def ag_table(l):
                nc.scalar.dma_start(tabmine[l][PC:PC + 1, :], dum_sb[:])
                nc.gpsimd.collective_compute(
                    "AllGather", mybir.AluOpType.bypass,
                    replica_groups=[list(range(NC))],
                    ins=[tabmine[l].opt()], outs=[tables[l].opt()])

            # ---------------- layer 0 node phase (own block only) --------
            half_t = (TILES + 1) // 2
            for h in range(2):
                r0, r1 = h * half_t, min((h + 1) * half_t, TILES)
                if r0 >= r1:
                    continue
                xin = npool.tile([f_in, half_t * 128], bf16, tag="xin")
                nc.sync.dma_start(xin[:, 0:(r1 - r0) * 128],
                                  xT[:, r0 * 128:r1 * 128])
                for r in range(r0, r1):
                    node_tile(0, r, xin[:, (r - r0) * 128:(r - r0 + 1) * 128])
            ag_table(0)

            for l in range(run_layers):
                table = tables[l]
                W = LOUT[l]
                sa = st_all[l]
                sa3 = sa[:].rearrange("p (r c) -> p r c", c=SW)

                # ---- bulk self/dst precompute (whole own block) ----
                # D12[r] = [exp(ad) x4 | exp(.2 ad) x4]
                d12 = sepool.tile([128, TILES * 8], bf16, tag="d12")
                d123 = d12[:].rearrange("p (r v) -> p r v", v=8)
                nc.scalar.activation(d123[:, :, 0:4], sa3[:, :, 64:68],
                                     ACT.Exp)
                nc.scalar.activation(d123[:, :, 4:8], sa3[:, :, 64:68],
                                     ACT.Exp, scale=0.2)
                # es[r] = max(E1*D1, E2*D2)  (self-loop numerator)
                est = sepool.tile([128, TILES * 8], bf16, tag="est")
                est3 = est[:].rearrange("p (r v) -> p r v", v=8)
                nc.vector.tensor_tensor(out=est3[:, :, :],
                                        in0=sa3[:, :, 68:76],
                                        in1=d123[:, :, :], op=ALU.mult)
                es_all = sepool.tile([128, TILES * 4], f32, tag="esal")
                es3 = es_all[:].rearrange("p (r q) -> p r q", q=4)
                nc.vector.tensor_tensor(out=es3[:, :, :],
                                        in0=est3[:, :, 0:4],
                                        in1=est3[:, :, 4:8], op=ALU.max)
                sv_all = sepool.tile([128, TILES * 64], bf16, tag="sval")
                sv4 = sv_all[:].rearrange("p (r q c) -> p r q c", q=4, c=16)
                nc.vector.tensor_tensor(
                    out=sv4[:, :, :, :],
                    in0=sa3[:, :, 0:64].rearrange("p r (q c) -> p r q c",
                                                  c=16),
                    in1=es3.unsqueeze(3).to_broadcast([128, TILES, 4, 16]),
                    op=ALU.mult)

                # ---------------- edge phase ----------------
                for pr in range(NPAIR):
                    rA = 2 * pr
                    rB = min(2 * pr + 1, TILES - 1)
                    single = rB == rA
                    tiles_here = [rA] if single else [rA, rB]
                    npr = len(tiles_here)
                    J = int(pair_J[pr])
                    coff, ncols = tile_cols[pr]

                    # per-tile ranges: (window_major_off, tile_major_off, k)
                    rngs_of = {}
                    JT = {}
                    for r in tiles_here:
                        o_t = 0
                        rr = []
                        for s in range(4):
                            o_w, k = tile_rng[r][s]
                            if k > 0:
                                rr.append((o_w, o_t, k))
                                o_t += k
                        rngs_of[r] = rr
                        JT[r] = o_t
                    base_t = {}
                    bt = 0
                    for r in tiles_here:
                        base_t[r] = bt
                        bt += JT[r]

                    if J > 0:
                        it = ipool.tile([128, max(ncols, 1)], i16, tag="idx")
                        nc.sync.dma_start(it[:, 0:ncols],
                                          idx_in[:, coff:coff + ncols])
                        gat = gpool.tile([128, J * 128], bf16, tag="gat")
                        g3 = gat[:].rearrange("p (j e) -> p j e", e=128)
                        ccol = 0
                        for (pr2, s_, joff, cj) in calls:
                            if pr2 != pr:
                                continue
                            n_i = 128 * cj
                            nc.gpsimd.dma_gather(
                                g3[:, joff:joff + cj, :],
                                table[s_ * WIN:(s_ + 1) * WIN, :],
                                it[:, ccol:ccol + n_i // 16],
                                n_i, n_i, 128,
                                queue_num=gather_queue(),
                                single_packet=False)
                            ccol += n_i // 16

                        gatj = gat[:].rearrange("p (j c) -> p j c", c=128)
                        g4 = gat[:].rearrange("p (j q c) -> p j q c",
                                              q=8, c=16)

                        # ---- t12 = E12_src * D12_dst (tile-major j) ----
                        t12 = epool.tile([128, J * 8], bf16, tag="t12")
                        t123 = t12[:].rearrange("p (j v) -> p j v", v=8)
                        for r in tiles_here:
                            d12r = d12[:, r * 8:(r + 1) * 8]
                            for (o_w, o_t, k) in rngs_of[r]:
                                to = base_t[r] + o_t
                                nc.vector.tensor_tensor(
                                    out=t123[:, to:to + k, :],
                                    in0=gatj[:, o_w:o_w + k, 68:76],
                                    in1=d12r.unsqueeze(1).to_broadcast(
                                        [128, k, 8]),
                                    op=ALU.mult)
                        # ---- e = max of halves (j-major [j][q]) ----
                        e_b = epool.tile([128, J * 4], bf16, tag="eb")
                        eb3 = e_b[:].rearrange("p (j q) -> p j q", q=4)
                        nc.vector.tensor_tensor(out=eb3[:, :, :],
                                                in0=t123[:, :, 0:4],
                                                in1=t123[:, :, 4:8],
                                                op=ALU.max)
                        # ---- denom per tile: strided reduce over j ----
                        ebqj = e_b[:].rearrange("p (j q) -> p q j", q=4)
                        den = epool.tile([128, 8], f32, tag="den")
                        for i, r in enumerate(tiles_here):
                            jt = JT[r]
                            if jt == 0:
                                continue
                            b0 = base_t[r]
                            nc.vector.tensor_reduce(
                                den[:, 4 * i:4 * i + 4],
                                ebqj[:, :, b0:b0 + jt], AX.X, ALU.add)
                        # ---- weighted values, tile-major bf16 ----
                        v_t = vpool.tile([128, J * 64], bf16, tag="vt")
                        v4 = v_t[:].rearrange("p (j q c) -> p j q c",
                                              q=4, c=16)
                        for r in tiles_here:
                            for (o_w, o_t, k) in rngs_of[r]:
                                to = base_t[r] + o_t
                                nc.vector.tensor_tensor(
                                    out=v4[:, to:to + k, :, :],
                                    in0=g4[:, o_w:o_w + k, 0:4, :],
                                    in1=eb3[:, to:to + k, :].unsqueeze(
                                        3).to_broadcast([128, k, 4, 16]),
                                    op=ALU.mult)
                        v3c = v_t[:].rearrange("p (j c) -> p c j", c=64)

                    # ---- pair-level aggregation + post ----
                    U = epool.tile([128, npr * 64], f32, tag="U")
                    dful = epool.tile([128, npr * 4], f32, tag="dful")
                    for i, r in enumerate(tiles_here):
                        jt = JT[r] if J > 0 else 0
                        if jt > 0:
                            b0 = base_t[r]
                            # in-place contiguous tree-fold (2x bf16 mode)
                            # down to <=4 slot columns, then strided tail
                            n = jt
                            while n > 4:
                                half = n // 2     # fold top half onto bottom
                                nc.vector.tensor_tensor(
                                    out=v_t[:, b0 * 64:(b0 + half) * 64],
                                    in0=v_t[:, b0 * 64:(b0 + half) * 64],
                                    in1=v_t[:, (b0 + n - half) * 64:
                                            (b0 + n) * 64],
                                    op=ALU.add)
                                n -= half
                            nc.vector.tensor_reduce(
                                U[:, 64 * i:64 * i + 64],
                                v3c[:, :, b0:b0 + n], AX.X, ALU.add)
                    all_j = J > 0 and all(JT[r] > 0 for r in tiles_here)
                    if all_j:
                        # contiguous pair slices (rB == rA + 1)
                        nc.vector.tensor_tensor(
                            out=U[:], in0=U[:],
                            in1=sv_all[:, rA * 64:(rA + npr) * 64],
                            op=ALU.add)
                        nc.vector.tensor_tensor(
                            out=dful[:], in0=den[:, 0:npr * 4],
                            in1=es_all[:, rA * 4:(rA + npr) * 4],
                            op=ALU.add)
                    else:
                        for i, r in enumerate(tiles_here):
                            jt = JT[r] if J > 0 else 0
                            if jt > 0:
                                nc.vector.tensor_tensor(
                                    out=U[:, 64 * i:64 * i + 64],
                                    in0=U[:, 64 * i:64 * i + 64],
                                    in1=sv_all[:, r * 64:(r + 1) * 64],
                                    op=ALU.add)
                                nc.vector.tensor_tensor(
                                    out=dful[:, 4 * i:4 * i + 4],
                                    in0=den[:, 4 * i:4 * i + 4],
                                    in1=es_all[:, r * 4:(r + 1) * 4],
                                    op=ALU.add)
                            else:
                                nc.vector.tensor_copy(
                                    U[:, 64 * i:64 * i + 64],
                                    sv_all[:, r * 64:(r + 1) * 64])
                                nc.vector.tensor_copy(
                                    dful[:, 4 * i:4 * i + 4],
                                    es_all[:, r * 4:(r + 1) * 4])
                    recip = epool.tile([128, npr * 4], f32, tag="rec")
                    nc.vector.reciprocal(recip[:], dful[:])
                    if l > 0:
                        nc.vector.tensor_scalar_mul(recip[:], recip[:], 0.25)
                    o64 = epool.tile([128, npr * 64], f32, tag="o64")
                    nc.vector.tensor_tensor(
                        out=o64[:].rearrange("p (t q c) -> p t q c",
                                             q=4, c=16),
                        in0=U[:].rearrange("p (t q c) -> p t q c",
                                           q=4, c=16),
                        in1=recip[:].rearrange(
                            "p (t q) -> p t q", q=4).unsqueeze(
                            3).to_broadcast([128, npr, 4, 16]),
                        op=ALU.mult)
                    if l == 0:
                        nc.vector.tensor_tensor(
                            out=o64[:], in0=o64[:],
                            in1=brep[0][:, 0:npr * 64], op=ALU.add)
                        elu_inplace(o64[:], npr * 64)
                        xnext = epool.tile([128, npr * 64], bf16, tag="xn")
                        nc.vector.tensor_copy(xnext[:], o64[:])
                        WX = 64
                    else:
                        o16 = epool.tile([128, npr * 16], f32, tag="o16")
                        nc.vector.tensor_reduce(
                            o16[:],
                            o64[:].rearrange("p (t q c) -> p t c q",
                                             q=4, c=16),
                            AX.X, ALU.add)
                        nc.vector.tensor_tensor(
                            out=o16[:], in0=o16[:],
                            in1=brep[l][:, 0:npr * 16], op=ALU.add)
                        if l == 1:
                            elu_inplace(o16[:], npr * 16)
                        xnext = epool.tile([128, npr * 16], bf16, tag="xn16")
                        nc.vector.tensor_copy(xnext[:], o16[:])
                        WX = 16

                    if l < 2:
                        for i, r in enumerate(tiles_here):
                            pst = ptpool.tile([WX, 128], bf16, tag="pst")
                            nc.tensor.transpose(
                                out=pst[:], in_=xnext[:, WX * i:WX * (i + 1)],
                                identity=ident[:])
                            stt = epool.tile([WX, 128], bf16, tag="stt")
                            nc.scalar.copy(stt[:], pst[:])
                            if run_layers > l + 1:
                                node_tile(l + 1, r, stt[:])
                    else:
                        for i, r in enumerate(tiles_here):
                            gt = epool.tile([128, G], bf16, tag="goh")
                            nc.sync.dma_start(
                                gt[:], goh_in[r * 128:(r + 1) * 128, :])
                            nc.tensor.matmul(
                                pool_ps[:], xnext[:, 16 * i:16 * i + 16],
                                gt[:], start=(r == 0),
                                stop=(r == TILES - 1))

                if l < 2 and run_layers > l + 1:
                    ag_table(l + 1)

            # ---------------- pooling + MLP head ----------------
            if run_layers == 3:
                pooled = hpool.tile([16, G], f32, tag="pooled")
                nc.scalar.copy(pooled[:], pool_ps[:])
                nc.sync.dma_start(cc_in[:, :], pooled[:])
                nc.gpsimd.collective_compute(
                    "AllReduce", mybir.AluOpType.add,
                    replica_groups=[list(range(NC))],
                    ins=[cc_in.opt()], outs=[cc_out.opt()])
                zt = hpool.tile([32, G], f32, tag="zt")
                nc.sync.dma_start(zt[0:16, :], cc_out[:, :])
                cr = hpool.tile([16, G], f32, tag="cr")
                nc.sync.dma_start(cr[:], cntr[:, :])
                nc.vector.tensor_tensor(out=zt[0:16, :], in0=zt[0:16, :],
                                        in1=cr[:], op=ALU.mult)
                nc.sync.dma_start(zt[16:32, :], statsT[:, :])
                fw1s = hpool.tile([32, 32], f32, tag="fw1")
                nc.sync.dma_start(fw1s[:], fw1[:, :])
                fb1s = hpool.tile([32, 1], f32, tag="fb1")
                nc.sync.dma_start(fb1s[:], fb1[:, :])
                fw2s = hpool.tile([32, 16], f32, tag="fw2")
                nc.sync.dma_start(fw2s[:], fw2[:, :])
                fb2s = hpool.tile([16, 1], f32, tag="fb2")
                nc.sync.dma_start(fb2s[:], fb2[:, :])
                fw3s = hpool.tile([16, 1], f32, tag="fw3")
                nc.sync.dma_start(fw3s[:], fw3[:, :])
                fb3s = hpool.tile([1, 1], f32, tag="fb3")
                nc.sync.dma_start(fb3s[:], fb3[:, :])

                mp1 = mpool.tile([32, G], f32, tag="mp1")
                nc.tensor.matmul(mp1[:], fw1s[:], zt[:], start=True, stop=True)
                h1 = hpool.tile([32, G], f32, tag="h1")
                nc.scalar.activation(h1[:], mp1[:], ACT.Relu, bias=fb1s[:, 0:1])
                mp2 = mpool.tile([16, G], f32, tag="mp2")
                nc.tensor.matmul(mp2[:], fw2s[:], h1[:], start=True, stop=True)
                h2 = hpool.tile([16, G], f32, tag="h2")
                nc.scalar.activation(h2[:], mp2[:], ACT.Relu, bias=fb2s[:, 0:1])
                mp3 = mpool.tile([1, G], f32, tag="mp3")
                nc.tensor.matmul(mp3[:], fw3s[:], h2[:], start=True, stop=True)
                ot = hpool.tile([1, G], f32, tag="ot")
                nc.vector.tensor_tensor(
                    out=ot[:], in0=mp3[:],
                    in1=fb3s[:, 0:1].to_broadcast([1, G]), op=ALU.add)
                nc.sync.dma_start(out_t[:, :], ot[:])

    nc.finalize()
    return nc


# ------------------------------------------------------------------- driver

def run_gat(x, stats, W1, a1s, a1d, b1, W2, a2s, a2d, b2, W3, a3s, a3d, b3,
            fw1, fb1, fw2, fb2, fw3, fb3, edge_index, batch,
            trace=False, _cache={}):
    from concourse.bass_utils import run_bass_kernel_spmd

    x = np.asarray(x, np.float32)
    stats = np.asarray(stats, np.float32)
    n_graphs = stats.shape[0]
    f_in = x.shape[1]
    meta = _prep(x, np.asarray(edge_index), np.asarray(batch), n_graphs)
    NC, PC, NSTAR = meta["NC"], meta["PC"], meta["NSTAR"]

    nc = _build(meta, n_graphs, f_in)

    # host-side input prep
    inv_pi = meta["inv_pi"]
    xs = np.zeros((NSTAR, f_in), np.float32)
    xs[:x.shape[0]] = x
    xT_full = np.ascontiguousarray(xs[inv_pi].T).astype(BF16)  # [f_in, NSTAR]

    cntrep = np.tile((1.0 / meta["counts"]).astype(np.float32)[None, :],
                     (16, 1))
    in_common = dict(
        w1=_augment_w(np.asarray(W1, np.float32), np.asarray(a1s, np.float32),
                      np.asarray(a1d, np.float32)),
        w2=_augment_w(np.asarray(W2, np.float32), np.asarray(a2s, np.float32),
                      np.asarray(a2d, np.float32)),
        w3=_augment_w(np.asarray(W3, np.float32), np.asarray(a3s, np.float32),
                      np.asarray(a3d, np.float32)),
        b1r=np.tile(np.asarray(b1, np.float32)[None, :], (128, 2)),
        b2r=np.tile(np.asarray(b2, np.float32)[None, :], (128, 2)),
        b3r=np.tile(np.asarray(b3, np.float32)[None, :], (128, 2)),
        cntr=cntrep.astype(np.float32),
        statsT=np.ascontiguousarray(stats.T).astype(np.float32),
        fw1=np.asarray(fw1, np.float32),
        fb1=np.asarray(fb1, np.float32).reshape(32, 1),
        fw2=np.asarray(fw2, np.float32),
        fb2=np.asarray(fb2, np.float32).reshape(16, 1),
        fw3=np.asarray(fw3, np.float32),
        fb3=np.asarray(fb3, np.float32).reshape(1, 1),
        dumr=np.zeros((1, 128), np.float32).astype(BF16),
    )
    in_maps = []
    for c in range(NC):
        m = dict(in_common)
        m["xT"] = np.ascontiguousarray(xT_full[:, c * PC:(c + 1) * PC])
        m["idx"] = np.ascontiguousarray(meta["idx_all"][c])
        m["goh"] = meta["goh"][c].astype(BF16)
        in_maps.append(m)

    res = run_bass_kernel_spmd(nc, in_maps, list(range(NC)), trace=trace)
    out = res.results[0]["out"]                      # [1, G]
    return np.ascontiguousarray(out.T).astype(np.float32), res


def kernel(**inputs):
    out, _ = run_gat(**inputs)
    return out


# revision 23
# speedup vs baseline: 1.2627x; 1.0938x over previous
"""GAT (3-layer, PyG-style) forward on 8 Trainium2 NeuronCores.

v3 strategy:
  - Node space padded to 8*PC nodes; core c owns nodes [c*PC, (c+1)*PC).
  - Per layer, a DRAM "table" holds one 256B row per node:
      [h 64 | AD 4 | E1 4 | E2 4 | pad] (bf16), E1=exp(as), E2=exp(0.2*as).
    Each core computes ONLY its own block (fused into the previous layer's
    edge loop); blocks are AllGathered (in two row-halves, overlapping the
    edge loop tail) into the full table.
  - exp(leaky_relu(as+ad)) == max(exp(as)exp(ad), exp(.2as)exp(.2ad)), so
    the edge phase needs no leaky/exp at all: gather rows, multiply the
    E12 columns by the dst's D12=(exp(ad),exp(.2ad)), take the max of the
    halves -> per-edge softmax numerators, one strided reduce per tile for
    the denominator, value mult (bf16) + one strided reduce per tile for
    the aggregation. Self contributions precomputed in bulk per layer.
  - Per-pair post-processing (softmax divide, bias, elu, head-mean,
    transpose + next-layer matmul) batches both tiles of a pair.
  - Final: per-tile pooling matmul into PSUM, AllReduce, MLP head.
"""

import sys

sys.path.insert(0, "/opt/trn_rl_repo")

import numpy as np
import ml_dtypes

BF16 = ml_dtypes.bfloat16

MAXJ_CALL = 28  # <=3584 idx per dma_gather (ring 4096 w/ 64KB scratch)
NQ = 4          # SWDGE queues


# ----------------------------------------------------------------- host prep

def _prep(x, edge_index, batch, n_graphs):
    """Graph preprocessing. Returns a dict of host arrays + structure."""
    N = x.shape[0]
    NC = 8
    PC = int(np.ceil(N / NC / 128)) * 128          # nodes per core (padded)
    NSTAR = NC * PC
    TILES = PC // 128
    BLKROWS = PC + 1                               # +1 dummy row per core blk
    WIN = 2 * BLKROWS                              # gather window (2 blocks)
    assert WIN <= 32767

    src = edge_index[0].astype(np.int64)
    dst = edge_index[1].astype(np.int64)

    core_of = np.arange(NSTAR) // PC               # orig id -> core
    win_of = (core_of // 2).astype(np.int64)       # orig id -> window

    # per-dst in-degree per window (real edges only; self-loops added densely)
    degw = np.zeros((NSTAR, 4), np.int64)
    np.add.at(degw, (dst, win_of[src]), 1)

    # --- cluster nodes into tiles by window-degree profile ---------------
    rank_of = np.empty(NSTAR, np.int64)
    tile_K = np.zeros((NC, TILES, 4), np.int64)
    for c in range(NC):
        ids = np.arange(c * PC, (c + 1) * PC)
        prof = degw[ids]
        order = np.lexsort((prof[:, 3], prof[:, 2], prof[:, 1], prof[:, 0],
                            prof.max(1), -prof.sum(1)))
        rank_of[ids[order]] = np.arange(PC)
        ps = prof[order]
        tile_K[c] = np.array([ps[t * 128:(t + 1) * 128].max(0)
                              for t in range(TILES)])

    # cross-core round matching: sort tiles by (max desc, total desc), then
    # "snake" positions so pair (2i,2i+1) = sorted ranks (i, TILES-1-i) —
    # balances per-pair gather sizes while keeping pair tiles adjacent.
    snake = np.empty(TILES, np.int64)
    for s in range(TILES):
        snake[s] = 2 * s if s < (TILES + 1) // 2 else 2 * (TILES - 1 - s) + 1
    for c in range(NC):
        t_order = np.lexsort((-tile_K[c].sum(1), -tile_K[c].max(1)))
        newpos = np.empty(TILES, np.int64)
        newpos[t_order] = snake
        ids = np.arange(c * PC, (c + 1) * PC)
        r = rank_of[ids]
        rank_of[ids] = newpos[r // 128] * 128 + (r % 128)
        inv_snake = np.empty(TILES, np.int64)
        inv_snake[newpos[t_order]] = t_order
        tile_K[c] = tile_K[c][inv_snake]

    K_round = tile_K.max(axis=0)                   # [TILES, 4]

    table_row = core_of * BLKROWS + rank_of

    # --- slot lists --------------------------------------------------------
    dcore = core_of[dst]
    drank = rank_of[dst]
    dwin = win_of[src]
    order = np.lexsort((dwin, drank, dcore))
    src_o, dst_o = src[order], dst[order]
    dcore_o, drank_o, dwin_o = dcore[order], drank[order], dwin[order]
    loc_o = table_row[src_o] - dwin_o * WIN        # window-local row idx
    assert loc_o.min() >= 0 and loc_o.max() < WIN

    DUMMY_LOC = PC                                  # same local idx all windows
    slots = []
    for c in range(NC):
        core_slots = []
        for t in range(TILES):
            wslots = []
            for s in range(4):
                K = int(K_round[t, s])
                arr = np.full((128, K), DUMMY_LOC, np.int16) if K else \
                    np.zeros((128, 0), np.int16)
                wslots.append(arr)
            core_slots.append(wslots)
        slots.append(core_slots)
    kfill = np.zeros((NSTAR, 4), np.int64)
    p_all = drank_o % 128
    t_all = drank_o // 128
    for i in range(len(src_o)):
        c = dcore_o[i]
        t = t_all[i]
        s = dwin_o[i]
        p = p_all[i]
        k = kfill[dst_o[i], s]
        slots[c][t][s][p, k] = loc_o[i]
        kfill[dst_o[i], s] = k + 1

    # --- pair-merged gather call structure (uniform across cores) --------
    NPAIR = (TILES + 1) // 2
    calls = []            # (pr, s, pair_joff, cj)
    pair_J = np.zeros(NPAIR, np.int64)
    tile_rng = [[None] * 4 for _ in range(TILES)]   # (pair_joff, K) per window
    for pr in range(NPAIR):
        rA, rB = 2 * pr, min(2 * pr + 1, TILES - 1)
        single = rB == rA
        joff = 0
        for s in range(4):
            KA = int(K_round[rA, s])
            KB = 0 if single else int(K_round[rB, s])
            tile_rng[rA][s] = (joff, KA)
            if not single:
                tile_rng[rB][s] = (joff + KA, KB)
            K = KA + KB
            o = 0
            while o < K:
                cj = min(MAXJ_CALL, K - o)
                calls.append((pr, s, joff + o, cj))
                o += cj
            joff += K
        pair_J[pr] = joff

    # --- int16 wrapped idx arrays per core --------------------------------
    def wrap16(ix):                                 # [n] -> [128, n//16]
        a = ix.reshape(-1, 16).T
        return np.tile(a, (8, 1))

    tile_cols = []
    off = 0
    for pr in range(NPAIR):
        ncols = int(128 * pair_J[pr]) // 16
        tile_cols.append((off, ncols))
        off += ncols

    def pair_blocks(c, pr, s):
        rA, rB = 2 * pr, min(2 * pr + 1, TILES - 1)
        bA = slots[c][rA][s]
        if rB == rA:
            return bA
        return np.concatenate([bA, slots[c][rB][s]], axis=1)

    idx_cores = []
    for c in range(NC):
        parts = []
        for (pr, s, joff, cj) in calls:
            base = tile_rng[2 * pr][s][0]
            js = joff - base
            blk = pair_blocks(c, pr, s)[:, js:js + cj]   # [128, cj]
            ix = blk.T.reshape(-1).astype(np.int16)
            parts.append(wrap16(ix).astype(np.int16))
        idx_cores.append(np.concatenate(parts, axis=1)
                         if parts else np.zeros((128, 0), np.int16))
    idx_all = np.stack(idx_cores)                    # [NC, 128, TOTC]

    pi_of = core_of * PC + rank_of
    inv_pi = np.empty(NSTAR, np.int64)
    inv_pi[pi_of] = np.arange(NSTAR)

    batch_full = np.full(NSTAR, -1, np.int64)
    batch_full[:N] = batch
    goh = np.zeros((NC, PC, n_graphs), np.float32)
    for c in range(NC):
        b = batch_full[inv_pi[c * PC:(c + 1) * PC]]
        valid = b >= 0
        goh[c, np.arange(PC)[valid], b[valid]] = 1.0
    counts = np.maximum(np.bincount(batch, minlength=n_graphs), 1.0)

    return dict(
        N=N, NC=NC, PC=PC, NSTAR=NSTAR, TILES=TILES, BLKROWS=BLKROWS,
        WIN=WIN, K_round=K_round, calls=calls,
        tile_cols=tile_cols, idx_all=idx_all, inv_pi=inv_pi,
        goh=goh, counts=counts, DUMMY_LOC=DUMMY_LOC,
        NPAIR=NPAIR, pair_J=pair_J, tile_rng=tile_rng,
    )


def _augment_w(W, a_s, a_d, heads=4, hid=16):
    """[F, H*C] weights -> [F, 72] augmented (bf16): [W | Wad | Was]."""
    F = W.shape[0]
    Wr = W.reshape(F, heads, hid)
    was = np.einsum("fhc,hc->fh", Wr, a_s)
    wad = np.einsum("fhc,hc->fh", Wr, a_d)
    out = np.concatenate([W, wad, was], axis=1).astype(np.float32)
    return out.astype(BF16)


# ------------------------------------------------------------- kernel build

def _build(meta, n_graphs, f_in, run_layers=3):
    import concourse.bass as bass
    import concourse.tile as tile
    from concourse import bacc, mybir
    from concourse.masks import make_identity

    NC, PC, TILES = meta["NC"], meta["PC"], meta["TILES"]
    BLKROWS, WIN = meta["BLKROWS"], meta["WIN"]
    TROWS = NC * BLKROWS
    calls, tile_cols = meta["calls"], meta["tile_cols"]
    NPAIR, pair_J, tile_rng = meta["NPAIR"], meta["pair_J"], meta["tile_rng"]
    TOTC = meta["idx_all"].shape[2]
    G = n_graphs
    f32, bf16, i16 = mybir.dt.float32, mybir.dt.bfloat16, mybir.dt.int16
    AX, ALU = mybir.AxisListType, mybir.AluOpType
    ACT = mybir.ActivationFunctionType

    nc = bacc.Bacc(None, target_bir_lowering=False, debug=False,
                   num_devices=NC, num_swdge_queues=NQ,
                   dynamic_dma_scratch_size=65536)

    # ---- I/O ----
    xT = nc.dram_tensor("xT", [f_in, PC], bf16, kind="ExternalInput")
    idx_in = nc.dram_tensor("idx", [128, TOTC], i16, kind="ExternalInput")
    goh_in = nc.dram_tensor("goh", [PC, G], bf16, kind="ExternalInput")
    w1 = nc.dram_tensor("w1", [f_in, 72], bf16, kind="ExternalInput")
    w2 = nc.dram_tensor("w2", [64, 72], bf16, kind="ExternalInput")
    w3 = nc.dram_tensor("w3", [16, 72], bf16, kind="ExternalInput")
    b1r = nc.dram_tensor("b1r", [128, 128], f32, kind="ExternalInput")
    b2r = nc.dram_tensor("b2r", [128, 32], f32, kind="ExternalInput")
    b3r = nc.dram_tensor("b3r", [128, 32], f32, kind="ExternalInput")
    cntr = nc.dram_tensor("cntr", [16, G], f32, kind="ExternalInput")
    statsT = nc.dram_tensor("statsT", [16, G], f32, kind="ExternalInput")
    fw1 = nc.dram_tensor("fw1", [32, 32], f32, kind="ExternalInput")
    fb1 = nc.dram_tensor("fb1", [32, 1], f32, kind="ExternalInput")
    fw2 = nc.dram_tensor("fw2", [32, 16], f32, kind="ExternalInput")
    fb2 = nc.dram_tensor("fb2", [16, 1], f32, kind="ExternalInput")
    fw3 = nc.dram_tensor("fw3", [16, 1], f32, kind="ExternalInput")
    fb3 = nc.dram_tensor("fb3", [1, 1], f32, kind="ExternalInput")
    dumr = nc.dram_tensor("dumr", [1, 128], bf16, kind="ExternalInput")
    out_t = nc.dram_tensor("out", [1, G], f32, kind="ExternalOutput")

    LIN = [f_in, 64, 16]          # node-phase input width per layer
    LOUT = [64, 16, 16]           # edge-phase output width per layer

    HROWS = (TILES // 2) * 128    # first-half rows for the split AllGather

    with tile.TileContext(nc, num_cores=NC) as tc:
        with (
            tc.tile_pool(name="dram", bufs=1, space="DRAM") as dpool,
            tc.tile_pool(name="consts", bufs=1) as cpool,
            tc.tile_pool(name="nodein", bufs=1) as npool,
            tc.tile_pool(name="psum", bufs=2, space="PSUM") as ppool,
            tc.tile_pool(name="pst", bufs=2, space="PSUM") as ptpool,
            tc.tile_pool(name="mlpp", bufs=1, space="PSUM") as mpool,
            tc.tile_pool(name="stall", bufs=1) as stpool,
            tc.tile_pool(name="selfp", bufs=1) as sepool,
            tc.tile_pool(name="gat", bufs=3) as gpool,
            tc.tile_pool(name="vt", bufs=2) as vpool,
            tc.tile_pool(name="idxp", bufs=2) as ipool,
            tc.tile_pool(name="edge", bufs=3) as epool,
            tc.tile_pool(name="escr", bufs=2) as e1pool,
            tc.tile_pool(name="poolacc", bufs=1, space="PSUM") as papool,
            tc.tile_pool(name="head", bufs=1) as hpool,
        ):
            tabmine = [dpool.tile([BLKROWS, 128], bf16, tag=f"tm{l}",
                                  name=f"tabmine{l}") for l in range(3)]
            tables = [dpool.tile([TROWS, 128], bf16, tag=f"tab{l}",
                                 name=f"table{l}", addr_space="Shared")
                      for l in range(3)]
            cc_in = dpool.tile([16, G], f32, tag="ccin")
            cc_out = dpool.tile([16, G], f32, tag="ccout",
                                addr_space="Shared")

            ident = cpool.tile([128, 128], bf16)
            make_identity(nc, ident[:])
            wsb = []
            for l, wt in enumerate((w1, w2, w3)):
                t = cpool.tile([LIN[l], 72], bf16, tag=f"w{l}", name=f"wsb{l}")
                nc.sync.dma_start(t[:], wt[:, :])
                wsb.append(t)
            brep = []
            for l, (bt, w2_) in enumerate(((b1r, 128), (b2r, 32), (b3r, 32))):
                t = cpool.tile([128, w2_], f32, tag=f"b{l}", name=f"bsb{l}")
                nc.sync.dma_start(t[:], bt[:, :])
                brep.append(t)
            dum_sb = cpool.tile([1, 128], bf16)
            nc.sync.dma_start(dum_sb[:], dumr[:, :])

            # own-block staged rows [h64|AD4|E1 4|E2 4] = 76 cols per tile
            SW = 76
            st_bufs = [stpool.tile([128, TILES * SW], bf16, tag=f"sta{l}",
                                   name=f"stall{l}") for l in range(2)]
            st_all = [st_bufs[0], st_bufs[1], st_bufs[0]]

            pool_ps = papool.tile([16, G], f32)
            goh_sb = stpool.tile([128, TILES * G], bf16, tag="gohsb")
            gview = goh_in[:].rearrange("(t p) g -> p t g", p=128)
            nc.sync.dma_start(
                goh_sb[:].rearrange("p (t g) -> p t g", g=G), gview)

            qctr = [0]

            def gather_queue():
                q = qctr[0] % NQ
                qctr[0] += 1
                return q

            def elu_inplace(x_ap, w):
                # elu(x) = relu(x) + exp(-relu(-x)) - 1
                t1 = e1pool.tile([128, w], bf16, tag="el1")
                t2 = e1pool.tile([128, w], bf16, tag="el2")
                nc.scalar.activation(t1[:], x_ap, ACT.Relu, scale=-1.0)
                nc.scalar.activation(t1[:], t1[:], ACT.Exp, scale=-1.0)
                nc.scalar.activation(t2[:], x_ap, ACT.Relu)
                nc.vector.tensor_scalar_add(t1[:], t1[:], -1.0)
                nc.vector.tensor_tensor(out=x_ap, in0=t1[:], in1=t2[:],
                                        op=ALU.add)

            def node_tile(l, r, lhsT_ap):
                """row_l[tile r] = [h|AD|E1|E2] -> st_all[l] slice + tabmine."""
                ps = ppool.tile([128, 72], f32, tag="nps")
                nc.tensor.matmul(ps[:], lhsT_ap, wsb[l][:],
                                 start=True, stop=True)
                sl = st_all[l][:, r * SW:(r + 1) * SW]
                nc.scalar.copy(sl[:, 0:68], ps[:, 0:68])
                nc.scalar.activation(sl[:, 68:72], ps[:, 68:72], ACT.Exp)
                nc.scalar.activation(sl[:, 72:76], ps[:, 68:72], ACT.Exp,
                                     scale=0.2)
                nc.scalar.dma_start(
                    tabmine[l][r * 128:(r + 1) * 128, 0:SW], sl)

            def ag_table(l):
                nc.scalar.dma_start(tabmine[l][PC:PC + 1, :], dum_sb[:])
                nc.gpsimd.collective_compute(
                    "AllGather", mybir.AluOpType.bypass,
                    replica_groups=[list(range(NC))],
                    ins=[tabmine[l].opt()], outs=[tables[l].opt()])

            # ---------------- layer 0 node phase (own block only) --------
            half_t = (TILES + 3) // 4
            for h in range(4):
                r0, r1 = h * half_t, min((h + 1) * half_t, TILES)
                if r0 >= r1:
                    continue
                xin = npool.tile([f_in, half_t * 128], bf16, tag="xin")
                nc.sync.dma_start(xin[:, 0:(r1 - r0) * 128],
                                  xT[:, r0 * 128:r1 * 128])
                for r in range(r0, r1):
                    node_tile(0, r, xin[:, (r - r0) * 128:(r - r0 + 1) * 128])
            ag_table(0)

            for l in range(run_layers):
                table = tables[l]
                W = LOUT[l]
                sa = st_all[l]
                sa3 = sa[:].rearrange("p (r c) -> p r c", c=SW)

                # ---- bulk self/dst precompute (whole own block) ----
                # D12[r] = [exp(ad) x4 | exp(.2 ad) x4]
                d12 = sepool.tile([128, TILES * 8], bf16, tag="d12")
                d123 = d12[:].rearrange("p (r v) -> p r v", v=8)
                nc.scalar.activation(d123[:, :, 0:4], sa3[:, :, 64:68],
                                     ACT.Exp)
                nc.scalar.activation(d123[:, :, 4:8], sa3[:, :, 64:68],
                                     ACT.Exp, scale=0.2)
                # es[r] = max(E1*D1, E2*D2)  (self-loop numerator)
                est = sepool.tile([128, TILES * 8], bf16, tag="est")
                est3 = est[:].rearrange("p (r v) -> p r v", v=8)
                nc.vector.tensor_tensor(out=est3[:, :, :],
                                        in0=sa3[:, :, 68:76],
                                        in1=d123[:, :, :], op=ALU.mult)
                es_all = sepool.tile([128, TILES * 4], f32, tag="esal")
                es3 = es_all[:].rearrange("p (r q) -> p r q", q=4)
                nc.vector.tensor_tensor(out=es3[:, :, :],
                                        in0=est3[:, :, 0:4],
                                        in1=est3[:, :, 4:8], op=ALU.max)
                sv_all = sepool.tile([128, TILES * 64], bf16, tag="sval")
                sv4 = sv_all[:].rearrange("p (r q c) -> p r q c", q=4, c=16)
                nc.vector.tensor_tensor(
                    out=sv4[:, :, :, :],
                    in0=sa3[:, :, 0:64].rearrange("p r (q c) -> p r q c",
                                                  c=16),
                    in1=es3.unsqueeze(3).to_broadcast([128, TILES, 4, 16]),
                    op=ALU.mult)

                # ---------------- edge phase ----------------
                for pr in range(NPAIR):
                    rA = 2 * pr
                    rB = min(2 * pr + 1, TILES - 1)
                    single = rB == rA
                    tiles_here = [rA] if single else [rA, rB]
                    npr = len(tiles_here)
                    J = int(pair_J[pr])
                    coff, ncols = tile_cols[pr]

                    # per-tile ranges: (window_major_off, tile_major_off, k)
                    rngs_of = {}
                    JT = {}
                    for r in tiles_here:
                        o_t = 0
                        rr = []
                        for s in range(4):
                            o_w, k = tile_rng[r][s]
                            if k > 0:
                                rr.append((o_w, o_t, k))
                                o_t += k
                        rngs_of[r] = rr
                        JT[r] = o_t
                    base_t = {}
                    bt = 0
                    for r in tiles_here:
                        base_t[r] = bt
                        bt += JT[r]

                    if J > 0:
                        it = ipool.tile([128, max(ncols, 1)], i16, tag="idx")
                        nc.sync.dma_start(it[:, 0:ncols],
                                          idx_in[:, coff:coff + ncols])
                        gat = gpool.tile([128, J * 128], bf16, tag="gat")
                        g3 = gat[:].rearrange("p (j e) -> p j e", e=128)
                        ccol = 0
                        for (pr2, s_, joff, cj) in calls:
                            if pr2 != pr:
                                continue
                            n_i = 128 * cj
                            nc.gpsimd.dma_gather(
                                g3[:, joff:joff + cj, :],
                                table[s_ * WIN:(s_ + 1) * WIN, :],
                                it[:, ccol:ccol + n_i // 16],
                                n_i, n_i, 128,
                                queue_num=gather_queue(),
                                single_packet=False)
                            ccol += n_i // 16

                        gatj = gat[:].rearrange("p (j c) -> p j c", c=128)
                        g4 = gat[:].rearrange("p (j q c) -> p j q c",
                                              q=8, c=16)

                        # ---- t12 = E12_src * D12_dst (tile-major j) ----
                        t12 = epool.tile([128, J * 8], bf16, tag="t12")
                        t123 = t12[:].rearrange("p (j v) -> p j v", v=8)
                        for r in tiles_here:
                            d12r = d12[:, r * 8:(r + 1) * 8]
                            for (o_w, o_t, k) in rngs_of[r]:
                                to = base_t[r] + o_t
                                nc.vector.tensor_tensor(
                                    out=t123[:, to:to + k, :],
                                    in0=gatj[:, o_w:o_w + k, 68:76],
                                    in1=d12r.unsqueeze(1).to_broadcast(
                                        [128, k, 8]),
                                    op=ALU.mult)
                        # ---- e = max of halves (j-major [j][q]) ----
                        e_b = epool.tile([128, J * 4], bf16, tag="eb")
                        eb3 = e_b[:].rearrange("p (j q) -> p j q", q=4)
                        nc.vector.tensor_tensor(out=eb3[:, :, :],
                                                in0=t123[:, :, 0:4],
                                                in1=t123[:, :, 4:8],
                                                op=ALU.max)
                        # ---- denom per tile: strided reduce over j ----
                        ebqj = e_b[:].rearrange("p (j q) -> p q j", q=4)
                        den = epool.tile([128, 8], f32, tag="den")
                        for i, r in enumerate(tiles_here):
                            jt = JT[r]
                            if jt == 0:
                                continue
                            b0 = base_t[r]
                            nc.vector.tensor_reduce(
                                den[:, 4 * i:4 * i + 4],
                                ebqj[:, :, b0:b0 + jt], AX.X, ALU.add)
                        # ---- weighted values, tile-major bf16 ----
                        v_t = vpool.tile([128, J * 64], bf16, tag="vt")
                        v4 = v_t[:].rearrange("p (j q c) -> p j q c",
                                              q=4, c=16)
                        for r in tiles_here:
                            for (o_w, o_t, k) in rngs_of[r]:
                                to = base_t[r] + o_t
                                nc.vector.tensor_tensor(
                                    out=v4[:, to:to + k, :, :],
                                    in0=g4[:, o_w:o_w + k, 0:4, :],
                                    in1=eb3[:, to:to + k, :].unsqueeze(
                                        3).to_broadcast([128, k, 4, 16]),
                                    op=ALU.mult)
                        v3c = v_t[:].rearrange("p (j c) -> p c j", c=64)

                    # ---- pair-level aggregation + post ----
                    U = epool.tile([128, npr * 64], f32, tag="U")
                    dful = epool.tile([128, npr * 4], f32, tag="dful")
                    for i, r in enumerate(tiles_here):
                        jt = JT[r] if J > 0 else 0
                        if jt > 0:
                            b0 = base_t[r]
                            # in-place contiguous tree-fold (2x bf16 mode)
                            # down to <=4 slot columns, then strided tail
                            n = jt
                            while n > 4:
                                half = n // 2     # fold top half onto bottom
                                nc.vector.tensor_tensor(
                                    out=v_t[:, b0 * 64:(b0 + half) * 64],
                                    in0=v_t[:, b0 * 64:(b0 + half) * 64],
                                    in1=v_t[:, (b0 + n - half) * 64:
                                            (b0 + n) * 64],
                                    op=ALU.add)
                                n -= half
                            nc.vector.tensor_reduce(
                                U[:, 64 * i:64 * i + 64],
                                v3c[:, :, b0:b0 + n], AX.X, ALU.add)
                    all_j = J > 0 and all(JT[r] > 0 for r in tiles_here)
                    if all_j:
                        # contiguous pair slices (rB == rA + 1)
                        nc.vector.tensor_tensor(
                            out=U[:], in0=U[:],
                            in1=sv_all[:, rA * 64:(rA + npr) * 64],
                            op=ALU.add)
                        nc.vector.tensor_tensor(
                            out=dful[:], in0=den[:, 0:npr * 4],
                            in1=es_all[:, rA * 4:(rA + npr) * 4],
                            op=ALU.add)
                    else:
                        for i, r in enumerate(tiles_here):
                            jt = JT[r] if J > 0 else 0
                            if jt > 0:
                                nc.vector.tensor_tensor(
                                    out=U[:, 64 * i:64 * i + 64],
                                    in0=U[:, 64 * i:64 * i + 64],
                                    in1=sv_all[:, r * 64:(r + 1) * 64],
                                    op=ALU.add)
                                nc.vector.tensor_tensor(
                                    out=dful[:, 4 * i:4 * i + 4],
                                    in0=den[:, 4 * i:4 * i + 4],
                                    in1=es_all[:, r * 4:(r + 1) * 4],
                                    op=ALU.add)
                            else:
                                nc.vector.tensor_copy(
                                    U[:, 64 * i:64 * i + 64],
                                    sv_all[:, r * 64:(r + 1) * 64])
                                nc.vector.tensor_copy(
                                    dful[:, 4 * i:4 * i + 4],
                                    es_all[:, r * 4:(r + 1) * 4])
                    recip = epool.tile([128, npr * 4], f32, tag="rec")
                    nc.vector.reciprocal(recip[:], dful[:])
                    if l > 0:
                        nc.vector.tensor_scalar_mul(recip[:], recip[:], 0.25)
                    o64 = epool.tile([128, npr * 64], f32, tag="o64")
                    nc.vector.tensor_tensor(
                        out=o64[:].rearrange("p (t q c) -> p t q c",
                                             q=4, c=16),
                        in0=U[:].rearrange("p (t q c) -> p t q c",
                                           q=4, c=16),
                        in1=recip[:].rearrange(
                            "p (t q) -> p t q", q=4).unsqueeze(
                            3).to_broadcast([128, npr, 4, 16]),
                        op=ALU.mult)
                    if l == 0:
                        nc.vector.tensor_tensor(
                            out=o64[:], in0=o64[:],
                            in1=brep[0][:, 0:npr * 64], op=ALU.add)
                        elu_inplace(o64[:], npr * 64)
                        xnext = epool.tile([128, npr * 64], bf16, tag="xn")
                        nc.vector.tensor_copy(xnext[:], o64[:])
                        WX = 64
                    else:
                        o16 = epool.tile([128, npr * 16], f32, tag="o16")
                        nc.vector.tensor_reduce(
                            o16[:],
                            o64[:].rearrange("p (t q c) -> p t c q",
                                             q=4, c=16),
                            AX.X, ALU.add)
                        nc.vector.tensor_tensor(
                            out=o16[:], in0=o16[:],
                            in1=brep[l][:, 0:npr * 16], op=ALU.add)
                        if l == 1:
                            elu_inplace(o16[:], npr * 16)
                        xnext = epool.tile([128, npr * 16], bf16, tag="xn16")
                        nc.vector.tensor_copy(xnext[:], o16[:])
                        WX = 16

                    if l < 2:
                        for i, r in enumerate(tiles_here):
                            pst = ptpool.tile([WX, 128], bf16, tag="pst")
                            nc.tensor.transpose(
                                out=pst[:], in_=xnext[:, WX * i:WX * (i + 1)],
                                identity=ident[:])
                            stt = epool.tile([WX, 128], bf16, tag="stt")
                            nc.scalar.copy(stt[:], pst[:])
                            if run_layers > l + 1:
                                node_tile(l + 1, r, stt[:])
                    else:
                        for i, r in enumerate(tiles_here):
                            nc.tensor.matmul(
                                pool_ps[:], xnext[:, 16 * i:16 * i + 16],
                                goh_sb[:, r * G:(r + 1) * G],
                                start=(r == 0),
                                stop=(r == TILES - 1))

                if l < 2 and run_layers > l + 1:
                    ag_table(l + 1)

            # ---------------- pooling + MLP head ----------------
            if run_layers == 3:
                pooled = hpool.tile([16, G], f32, tag="pooled")
                nc.scalar.copy(pooled[:], pool_ps[:])
                nc.sync.dma_start(cc_in[:, :], pooled[:])
                nc.gpsimd.collective_compute(
                    "AllReduce", mybir.AluOpType.add,
                    replica_groups=[list(range(NC))],
                    ins=[cc_in.opt()], outs=[cc_out.opt()])
                zt = hpool.tile([32, G], f32, tag="zt")
                nc.sync.dma_start(zt[0:16, :], cc_out[:, :])
                cr = hpool.tile([16, G], f32, tag="cr")
                nc.sync.dma_start(cr[:], cntr[:, :])
                nc.vector.tensor_tensor(out=zt[0:16, :], in0=zt[0:16, :],
                                        in1=cr[:], op=ALU.mult)
                nc.sync.dma_start(zt[16:32, :], statsT[:, :])
                fw1s = hpool.tile([32, 32], f32, tag="fw1")
                nc.sync.dma_start(fw1s[:], fw1[:, :])
                fb1s = hpool.tile([32, 1], f32, tag="fb1")
                nc.sync.dma_start(fb1s[:], fb1[:, :])
                fw2s = hpool.tile([32, 16], f32, tag="fw2")
                nc.sync.dma_start(fw2s[:], fw2[:, :])
                fb2s = hpool.tile([16, 1], f32, tag="fb2")
                nc.sync.dma_start(fb2s[:], fb2[:, :])
                fw3s = hpool.tile([16, 1], f32, tag="fw3")
                nc.sync.dma_start(fw3s[:], fw3[:, :])
                fb3s = hpool.tile([1, 1], f32, tag="fb3")
                nc.sync.dma_start(fb3s[:], fb3[:, :])

                mp1 = mpool.tile([32, G], f32, tag="mp1")
                nc.tensor.matmul(mp1[:], fw1s[:], zt[:], start=True, stop=True)
                h1 = hpool.tile([32, G], f32, tag="h1")
                nc.scalar.activation(h1[:], mp1[:], ACT.Relu, bias=fb1s[:, 0:1])
                mp2 = mpool.tile([16, G], f32, tag="mp2")
                nc.tensor.matmul(mp2[:], fw2s[:], h1[:], start=True, stop=True)
                h2 = hpool.tile([16, G], f32, tag="h2")
                nc.scalar.activation(h2[:], mp2[:], ACT.Relu, bias=fb2s[:, 0:1])
                mp3 = mpool.tile([1, G], f32, tag="mp3")
                nc.tensor.matmul(mp3[:], fw3s[:], h2[:], start=True, stop=True)
                ot = hpool.tile([1, G], f32, tag="ot")
                nc.vector.tensor_tensor(
                    out=ot[:], in0=mp3[:],
                    in1=fb3s[:, 0:1].to_broadcast([1, G]), op=ALU.add)
                nc.sync.dma_start(out_t[:, :], ot[:])

    nc.finalize()
    return nc


# ------------------------------------------------------------------- driver

def run_gat(x, stats, W1, a1s, a1d, b1, W2, a2s, a2d, b2, W3, a3s, a3d, b3,
            fw1, fb1, fw2, fb2, fw3, fb3, edge_index, batch,
            trace=False, _cache={}):
    from concourse.bass_utils import run_bass_kernel_spmd

    x = np.asarray(x, np.float32)
    stats = np.asarray(stats, np.float32)
    n_graphs = stats.shape[0]
    f_in = x.shape[1]
    meta = _prep(x, np.asarray(edge_index), np.asarray(batch), n_graphs)
    NC, PC, NSTAR = meta["NC"], meta["PC"], meta["NSTAR"]

    nc = _build(meta, n_graphs, f_in)

    # host-side input prep
    inv_pi = meta["inv_pi"]
    xs = np.zeros((NSTAR, f_in), np.float32)
    xs[:x.shape[0]] = x
    xT_full = np.ascontiguousarray(xs[inv_pi].T).astype(BF16)  # [f_in, NSTAR]

    cntrep = np.tile((1.0 / meta["counts"]).astype(np.float32)[None, :],
                     (16, 1))
    in_common = dict(
        w1=_augment_w(np.asarray(W1, np.float32), np.asarray(a1s, np.float32),
                      np.asarray(a1d, np.float32)),
        w2=_augment_w(np.asarray(W2, np.float32), np.asarray(a2s, np.float32),
                      np.asarray(a2d, np.float32)),
        w3=_augment_w(np.asarray(W3, np.float32), np.asarray(a3s, np.float32),
                      np.asarray(a3d, np.float32)),
        b1r=np.tile(np.asarray(b1, np.float32)[None, :], (128, 2)),
        b2r=np.tile(np.asarray(b2, np.float32)[None, :], (128, 2)),
        b3r=np.tile(np.asarray(b3, np.float32)[None, :], (128, 2)),
        cntr=cntrep.astype(np.float32),
        statsT=np.ascontiguousarray(stats.T).astype(np.float32),
        fw1=np.asarray(fw1, np.float32),
        fb1=np.asarray(fb1, np.float32).reshape(32, 1),
        fw2=np.asarray(fw2, np.float32),
        fb2=np.asarray(fb2, np.float32).reshape(16, 1),
        fw3=np.asarray(fw3, np.float32),
        fb3=np.asarray(fb3, np.float32).reshape(1, 1),
        dumr=np.zeros((1, 128), np.float32).astype(BF16),
    )
    in_maps = []
    for c in range(NC):
        m = dict(in_common)
        m["xT"] = np.ascontiguousarray(xT_full[:, c * PC:(c + 1) * PC])
        m["idx"] = np.ascontiguousarray(meta["idx_all"][c])
        m["goh"] = meta["goh"][c].astype(BF16)
        in_maps.append(m)

    res = run_bass_kernel_spmd(nc, in_maps, list(range(NC)), trace=trace)
    out = res.results[0]["out"]                      # [1, G]
    return np.ascontiguousarray(out.T).astype(np.float32), res


def kernel(**inputs):
    out, _ = run_gat(**inputs)
    return out
